# revision 30
# baseline (speedup 1.0000x reference)
"""Trainium2 Bass kernel for nn_CrossAttention (N=65536 gaussians, M=512 tokens, D=512).

Runs SPMD on 8 NeuronCores; N sharded across cores.

Host marshalling supplies g/g_p in both natural layout (f32) and transposed
packed layout ([128, nt, 4, 128]; f32 for the pooling pass, bf16 for the
attention pass), plus pre-transposed weight packs.  This removes all on-device
PE transposes and their PSUM->SBUF copies.

Per core (n_loc rows):
  Phase A (pool): per 128-row tile, p_s = g @ Ws.T via host-gT stationary;
    et = exp(p_s - C_SHIFT); accumulate P.T = gp.T @ et and l = 1.T @ et in
    PSUM across all tiles.
  AllReduce (bf16) of (P.T || l); weight-prep matmuls (K0T, V, Aq0, Agp0,
    GqT, GgpT, c0, u0) are emitted after phase A so they execute during the
    collective.
  Fixups: kplT = P.T/l;  Aq = SCALE*Wq.T@K.T = Aq0 + GqT@kplT (stored bf16),
    likewise Agp; c = c0 + u0-part (score bias from bq+bgp; zero for the
    given inputs but handled generally).
  Phase B (attention): per 512-row chunk, scoresT = Aq.T@gT + Agp.T@gpT in
    bf16; ep = exp(scoresT + c); OV and rowsum r via PE; LN fused:
    pre = OV*(1/r) + g (scalar_tensor_tensor, accum -> mean),
    sumsq via activation Square accum, rstd = exp(-0.5*ln(var+eps)),
    out = (pre - mu)*rstd [*gamma + beta if non-trivial].
"""
import numpy as np
from ml_dtypes import bfloat16

import concourse.bass as bass
import concourse.tile as tile
from concourse import bacc, bass_isa, mybir, bass_utils

N_CORES = 8
N_FULL = 65536
D = 512
M = 512
SCALE = (D // 8) ** -0.5  # 0.125
LN_EPS = 1e-5
C_SHIFT = 115.0
F32 = mybir.dt.float32
F32R = mybir.dt.float32r
BF16 = mybir.dt.bfloat16
EXP = mybir.ActivationFunctionType.Exp
LN_F = mybir.ActivationFunctionType.Ln
SQUARE = mybir.ActivationFunctionType.Square
MULT = mybir.AluOpType.mult
ADD = mybir.AluOpType.add
SUB = mybir.AluOpType.subtract


def _bcast(ap, parts):
    """Partition-broadcast a [F]-shaped DRAM AP to [parts, F] for DMA."""
    return bass.AP(tensor=ap.tensor, offset=ap.offset, ap=[[0, parts], *ap.ap])


def build(n_loc=N_FULL // N_CORES, n_cores=N_CORES, gb_trivial=True):
    nt = n_loc // 128      # phase-A tiles of 128 rows
    nch = n_loc // 512     # phase-B chunks of 512 rows
    assert n_loc % 512 == 0

    nc = bacc.Bacc("TRN2", target_bir_lowering=False, debug=False, num_devices=n_cores)
    gt32_d = nc.dram_tensor("gt32", [128, nt, 4, 128], F32R, kind="ExternalInput").ap()
    gp32_d = nc.dram_tensor("gp32", [n_loc, D], F32R, kind="ExternalInput").ap()
    gtb_d = nc.dram_tensor("gtb", [128, nt, 4, 128], BF16, kind="ExternalInput").ap()
    gptb_d = nc.dram_tensor("gptb", [128, nt, 4, 128], BF16, kind="ExternalInput").ap()
    gres_d = nc.dram_tensor("gres", [n_loc, D], F32, kind="ExternalInput").ap()
    wsT_d = nc.dram_tensor("wsT", [128, 4, D], F32R, kind="ExternalInput").ap()
    wkT_d = nc.dram_tensor("wkT", [128, 4, D], F32R, kind="ExternalInput").ap()
    wvT_d = nc.dram_tensor("wvT", [128, 4, D], F32R, kind="ExternalInput").ap()
    wq_d = nc.dram_tensor("wq", [D, D], F32R, kind="ExternalInput").ap()
    wgp_d = nc.dram_tensor("wgp", [D, D], F32R, kind="ExternalInput").ap()
    wkp_d = nc.dram_tensor("wkp", [D, D], F32R, kind="ExternalInput").ap()
    bq_d = nc.dram_tensor("bq", [D], F32, kind="ExternalInput").ap()
    bk_d = nc.dram_tensor("bk", [D], F32, kind="ExternalInput").ap()
    bv_d = nc.dram_tensor("bv", [D], F32, kind="ExternalInput").ap()
    bgp_d = nc.dram_tensor("bgp", [D], F32, kind="ExternalInput").ap()
    bkp_d = nc.dram_tensor("bkp", [D], F32, kind="ExternalInput").ap()
    if not gb_trivial:
        gam_d = nc.dram_tensor("gamma", [D], F32, kind="ExternalInput").ap()
        bet_d = nc.dram_tensor("beta", [D], F32, kind="ExternalInput").ap()
    out_d = nc.dram_tensor("out", [n_loc, D], F32, kind="ExternalOutput").ap()

    with tile.TileContext(nc) as tc:
        with (
            tc.tile_pool(name="wts", bufs=1) as wts,
            tc.tile_pool(name="ps", bufs=3, space="PSUM") as pps,
            tc.tile_pool(name="dram", bufs=1, space="DRAM") as dpool,
        ):
            # ---------- persistent tiles ----------
            # Pin the activation table to set 6 (natural_log_exp_and_others:
            # exp+ln+copy+square) so the table-load pass never alternates
            # between the exp-first and ln-first sets (1.28us per reload).
            nc.scalar.add_instruction(mybir.InstLoadActFuncSet(
                name=nc.get_next_instruction_name(),
                act_func_set_id=6, ins=[], outs=[]))
            ones_f = wts.tile([128, 128], F32)
            nc.vector.memset(ones_f, 1.0)
            ones_m = wts.tile([128, 128], F32R)
            nc.vector.tensor_copy(out=ones_m, in_=ones_f)
            ones_c = wts.tile([128, 2], F32R)
            nc.vector.tensor_copy(out=ones_c, in_=ones_f[:, 0:2])
            negc_sb = wts.tile([128, 1], F32)
            nc.vector.memset(negc_sb, -C_SHIFT)
            eps_sb = wts.tile([128, 1], F32)
            nc.vector.memset(eps_sb, LN_EPS)
            nhalf_sb = wts.tile([128, 1], F32)
            nc.vector.memset(nhalf_sb, -0.5)
            wsT = wts.tile([128, 4, D], F32R)
            for k in range(4):
                nc.sync.dma_start(out=wsT[:, k, :], in_=wsT_d[:, k, :])
            v_sb = wts.tile([128, 4, D], F32R)   # V [m-part, mt, d]
            aq_sb = wts.tile([128, 4, D], BF16)  # SCALE*Wq.T@K.T [d-part, dt, m]
            agp_sb = wts.tile([128, 4, D], BF16)
            c_sb = wts.tile([128, 8], F32)       # score bias c[m] as [m-part, (mt,2)]
            bv_bc = wts.tile([128, D], F32)
            nc.scalar.dma_start(out=bv_bc, in_=_bcast(bv_d, 128))
            if not gb_trivial:
                gam_bc = wts.tile([128, D], F32)
                bet_bc = wts.tile([128, D], F32)
                nc.scalar.dma_start(out=gam_bc, in_=_bcast(gam_d, 128))
                nc.scalar.dma_start(out=bet_bc, in_=_bcast(bet_d, 128))
            # weight/bias tiles persist in wts; their DMAs ride the Act-engine
            # DGE queue and are emitted a few tiles into phase A so the first
            # g-tile loads aren't bandwidth-starved.
            wkT = wts.tile([128, 4, D], F32R)
            wvT = wts.tile([128, 4, D], F32R)
            wq_n = wts.tile([128, 4, D], F32R)
            wgp_n = wts.tile([128, 4, D], F32R)
            wkp_n = wts.tile([128, 4, D], F32R)

            def _weight_dmas():
                nc.scalar.dma_start(out=wkT, in_=wkT_d)
                nc.scalar.dma_start(out=wvT, in_=wvT_d)
                for t_sb, t_d in [(wq_n, wq_d), (wgp_n, wgp_d), (wkp_n, wkp_d)]:
                    nc.scalar.dma_start(
                        out=t_sb, in_=t_d.rearrange("(t p) d -> p t d", p=128))
            bk_sb = wts.tile([128, 4], F32)
            bs_sb = wts.tile([128, 4], F32)
            bq_sb = wts.tile([128, 4], F32)
            bkp_sb = wts.tile([128, 4], F32)
            nc.scalar.dma_start(out=bk_sb, in_=bk_d.rearrange("(t p) -> p t", p=128))
            nc.scalar.dma_start(out=bq_sb, in_=bq_d.rearrange("(t p) -> p t", p=128))
            nc.scalar.dma_start(out=bs_sb, in_=bgp_d.rearrange("(t p) -> p t", p=128))
            nc.scalar.dma_start(out=bkp_sb,
                                in_=bkp_d.rearrange("(t p) -> p t", p=128))

            # warm the PE pstate during the initial DMA wait
            p_w = pps.tile([128, 128], F32, tag="s", name="p_warm")
            for w in range(24):
                nc.tensor.matmul(p_w[:], ones_m[:], ones_m[:],
                                 start=(w == 0), stop=(w == 23))

            # ---------- phase A: pooling partials ----------
            ctxA = tc.tile_pool(name="pAacc", bufs=1, space="PSUM")
            pAacc = ctxA.__enter__()
            ps_p = [pAacc.tile([128, 512], F32, tag=f"psp{i}", bufs=1,
                               name=f"ps_p{i}") for i in range(4)]
            l_acc = wts.tile([128, 512], F32)
            nc.vector.memset(l_acc, 0.0)
            with tc.tile_pool(name="sA", bufs=4) as sA:
                for i in range(nt):
                    if i == 6:
                        _weight_dmas()
                    gT_t = sA.tile([128, 4, 128], F32R, tag="gTA", name=f"gTA{i}")
                    gpn = sA.tile([128, D], F32R, tag="gpnA", name=f"gpnA{i}")
                    nc.sync.dma_start(out=gT_t, in_=gt32_d[:, i, :, :])
                    nc.sync.dma_start(out=gpn, in_=gp32_d[i * 128:(i + 1) * 128, :])
                    p_s = pps.tile([128, 512], F32, tag="s", name=f"psA{i}")
                    for dk in range(4):
                        nc.tensor.matmul(p_s[:], gT_t[:, dk, :], wsT[:, dk, :],
                                         start=(dk == 0), stop=(dk == 3))
                    et = sA.tile([128, 512], F32R, tag="etA", name=f"etA{i}")
                    nc.scalar.activation(out=et, in_=p_s[:], func=EXP,
                                         bias=negc_sb, scale=1.0)
                    for d2t in range(4):
                        nc.tensor.matmul(
                            ps_p[d2t][:], gpn[:, d2t * 128:(d2t + 1) * 128], et[:],
                            start=(i == 0), stop=(i == nt - 1))
                    # l += colsums(et) off the PE: partition-reduce on the Q7
                    # (Pool) engine, accumulate on the (idle) DVE.
                    l_i = sA.tile([128, 512], F32, tag="liA", name=f"liA{i}")
                    nc.gpsimd.partition_all_reduce(
                        l_i, et[:], channels=128, reduce_op=bass_isa.ReduceOp.add)
                    nc.vector.tensor_add(out=l_acc, in0=l_acc, in1=l_i)

            # ---------- all-reduce of (P.T || l), bf16 ----------
            # pf opens BEFORE w0 so its SBUF region does not overlap w0's
            # tiles; its prefetch DMAs (sync queue) can then run during the
            # collective without anti-dependency stalls.
            pf_ctx = tc.tile_pool(name="pf", bufs=4)
            pf = pf_ctx.__enter__()
            pf_tiles = {}

            def _prefetch(c):
                gtb_c = pf.tile([128, 4, 4, 128], BF16, tag="gtb", name=f"gtb{c}")
                gptb_c = pf.tile([128, 4, 4, 128], BF16, tag="gptb", name=f"gptb{c}")
                gres_c = pf.tile([128, 4, 512], F32, tag="gres", name=f"gres{c}")
                nc.sync.dma_start(out=gtb_c, in_=gtb_d[:, 4 * c:4 * c + 4, :, :])
                nc.sync.dma_start(out=gptb_c, in_=gptb_d[:, 4 * c:4 * c + 4, :, :])
                nc.sync.dma_start(
                    out=gres_c,
                    in_=gres_d[c * 512:(c + 1) * 512, :].rearrange(
                        "(nk p) d -> p nk d", p=128))
                pf_tiles[c] = (gtb_c, gptb_c, gres_c)

            with tc.tile_pool(name="arp", bufs=1) as arp:
                pl_sb = arp.tile([128, 5, 512], BF16)
                for d2t in range(4):
                    if d2t % 2 == 0:
                        nc.vector.tensor_copy(out=pl_sb[:, d2t, :], in_=ps_p[d2t][:])
                    else:
                        nc.scalar.copy(out=pl_sb[:, d2t, :], in_=ps_p[d2t][:])
                nc.vector.tensor_copy(out=pl_sb[:, 4, :], in_=l_acc)
                ctxA.__exit__(None, None, None)
                # ReduceScatter + AllGather = AllReduce, but ~20% cheaper on
                # the interconnect (no duplicate-reduce traffic).
                ar_in = dpool.tile([128, 5 * 512], BF16)
                rs_out = dpool.tile([128, 5 * 512 // n_cores], BF16)
                ar_out = dpool.tile([128, 5 * 512], BF16, addr_space="Shared")
                nc.sync.dma_start(out=ar_in[:],
                                  in_=pl_sb[:].rearrange("p a b -> p (a b)"))
                nc.gpsimd.collective_compute(
                    "ReduceScatter", mybir.AluOpType.add,
                    replica_groups=[list(range(n_cores))],
                    ins=[ar_in.opt()], outs=[rs_out.opt()])
                nc.gpsimd.collective_compute(
                    "AllGather", mybir.AluOpType.bypass,
                    replica_groups=[list(range(n_cores))],
                    ins=[rs_out.opt()], outs=[ar_out.opt()])
                # phase-B prefetch begins immediately; these loads only await
                # free pf buffers, so they overlap the collective.
                for c in range(min(4, nch)):
                    _prefetch(c)

                # ---------- phase 0 weight prep (overlaps the collective) ----
                with tc.tile_pool(name="w0", bufs=1) as w0:
                    nc.vector.tensor_add(out=bs_sb, in0=bs_sb, in1=bq_sb)
                    # K.T = Wk@Ws.T + (bk + bkp) + Wkp@kpool.T -> fold bkp into K0T bias
                    nc.vector.tensor_add(out=bk_sb, in0=bk_sb, in1=bkp_sb)

                    k0T = w0.tile([128, 4, D], F32R)
                    aq0 = w0.tile([128, 4, D], F32)
                    agp0 = w0.tile([128, 4, D], F32)
                    gqT = w0.tile([128, 4, D], F32R)
                    ggpT = w0.tile([128, 4, D], F32R)
                    c0_sb = w0.tile([128, 8], F32)
                    u0c2 = w0.tile([128, 4, 2], F32R)
                    # K0T[d, m] = Wk @ Ws.T + bk'
                    for dt in range(4):
                        p_k = pps.tile([128, 512], F32, tag="s", name=f"pk{dt}")
                        for di in range(4):
                            nc.tensor.matmul(p_k[:], wkT[:, di, dt * 128:(dt + 1) * 128],
                                             wsT[:, di, :], start=(di == 0), stop=(di == 3))
                        nc.vector.tensor_scalar_add(out=k0T[:, dt, :], in0=p_k[:],
                                                    scalar1=bk_sb[:, dt:dt + 1])
                    # V[m, d] = Ws @ Wv.T + bv
                    for mt in range(4):
                        p_v = pps.tile([128, 512], F32, tag="s", name=f"pv{mt}")
                        for di in range(4):
                            nc.tensor.matmul(p_v[:], wsT[:, di, mt * 128:(mt + 1) * 128],
                                             wvT[:, di, :], start=(di == 0), stop=(di == 3))
                        nc.vector.tensor_add(out=v_sb[:, mt, :], in0=p_v[:], in1=bv_bc)
                    # Aq0 = SCALE*Wq.T@K0T ; Agp0 likewise
                    for w_nat, dst in [(wq_n, aq0), (wgp_n, agp0)]:
                        for dt in range(4):
                            p_a = pps.tile([128, 512], F32, tag="s",
                                           name=f"pa_{dst.tensor.name}_{dt}")
                            for di in range(4):
                                nc.tensor.matmul(
                                    p_a[:], w_nat[:, di, dt * 128:(dt + 1) * 128],
                                    k0T[:, di, :], start=(di == 0), stop=(di == 3))
                            nc.scalar.mul(out=dst[:, dt, :], in_=p_a[:], mul=SCALE)
                    # GqT = SCALE*(Wkp.T @ Wq) ; GgpT likewise
                    for w_nat, dst in [(wq_n, gqT), (wgp_n, ggpT)]:
                        for dt in range(4):
                            p_gq = pps.tile([128, 512], F32, tag="s",
                                            name=f"pg_{dst.tensor.name}_{dt}")
                            for di in range(4):
                                nc.tensor.matmul(
                                    p_gq[:], wkp_n[:, di, dt * 128:(dt + 1) * 128],
                                    w_nat[:, di, :], start=(di == 0), stop=(di == 3))
                            nc.scalar.mul(out=dst[:, dt, :], in_=p_gq[:], mul=SCALE)
                    # c0[m] = SCALE*(bq+bgp)@K0T ; u0 = SCALE*Wkp.T@(bq+bgp)
                    bsr2 = w0.tile([128, 4, 2], F32R)
                    nc.vector.tensor_copy(out=bsr2[:, :, 0], in_=bs_sb)
                    nc.vector.tensor_copy(out=bsr2[:, :, 1], in_=bs_sb)
                    ctx0 = tc.tile_pool(name="p0acc", bufs=1, space="PSUM")
                    p0acc = ctx0.__enter__()
                    p_c0 = p0acc.tile([128, 8], F32, tag="pc0", bufs=1, name="p_c0")
                    p_u0 = p0acc.tile([128, 8], F32, tag="pu0", bufs=1, name="p_u0")
                    for mt in range(4):
                        for di in range(4):
                            nc.tensor.matmul(
                                p_c0[:, mt * 2:mt * 2 + 2],
                                k0T[:, di, mt * 128:(mt + 1) * 128],
                                bsr2[:, di, :], start=(di == 0), stop=(di == 3))
                            nc.tensor.matmul(
                                p_u0[:, mt * 2:mt * 2 + 2],
                                wkp_n[:, di, mt * 128:(mt + 1) * 128],
                                bsr2[:, di, :], start=(di == 0), stop=(di == 3))
                    nc.scalar.mul(out=c0_sb, in_=p_c0[:], mul=SCALE)
                    nc.scalar.mul(out=u0c2.rearrange("p a b -> p (a b)"), in_=p_u0[:],
                                  mul=SCALE)
                    ctx0.__exit__(None, None, None)

                    # ---------- post-collective fixups ----------
                    # plr rides the Act-engine DGE queue (idle here), keeping
                    # the sync queue free for phase-B prefetch.
                    plr_sb = w0.tile([128, 5, 512], BF16)
                    nc.scalar.dma_start(out=plr_sb,
                                        in_=ar_out[:].rearrange("p (a b) -> p a b", a=5))
                    lr_sb = w0.tile([128, 512], F32)
                    nc.vector.reciprocal(out=lr_sb, in_=plr_sb[:, 4, :])
                    kplT = w0.tile([128, 4, D], F32R)
                    for dint in range(4):
                        nc.vector.tensor_mul(out=kplT[:, dint, :],
                                             in0=plr_sb[:, dint, :], in1=lr_sb)
                    for gT_w, base, dst in [(gqT, aq0, aq_sb), (ggpT, agp0, agp_sb)]:
                        for dt in range(4):
                            p_aq = pps.tile([128, 512], F32, tag="s",
                                            name=f"paq_{dst.tensor.name}_{dt}")
                            for di in range(4):
                                nc.tensor.matmul(
                                    p_aq[:], gT_w[:, di, dt * 128:(dt + 1) * 128],
                                    kplT[:, di, :], start=(di == 0), stop=(di == 3))
                            nc.vector.tensor_add(out=dst[:, dt, :], in0=base[:, dt, :],
                                                 in1=p_aq[:])
                    ctxP = tc.tile_pool(name="pPacc", bufs=1, space="PSUM")
                    pPacc = ctxP.__enter__()
                    p_cp = pPacc.tile([128, 8], F32, tag="pcp", bufs=1, name="p_cp")
                    for mt in range(4):
                        for di in range(4):
                            nc.tensor.matmul(
                                p_cp[:, mt * 2:mt * 2 + 2],
                                kplT[:, di, mt * 128:(mt + 1) * 128],
                                u0c2[:, di, :], start=(di == 0), stop=(di == 3))
                    nc.vector.tensor_add(out=c_sb, in0=c0_sb, in1=p_cp[:])
                    ctxP.__exit__(None, None, None)

            # ---------- phase B: attention ----------
            with (tc.tile_pool(name="eB", bufs=2) as eB,
                  tc.tile_pool(name="pBacc", bufs=1, space="PSUM") as pBacc):
                for c in range(nch):
                    if c + 4 < nch:
                        _prefetch(c + 4)
                    gtb_c, gptb_c, gres_c = pf_tiles.pop(c)
                    ps_ov = [pBacc.tile([128, 512], F32, tag=f"ov{k}", bufs=1,
                                        name=f"ov{c}_{k}") for k in range(4)]
                    ps_r = pBacc.tile([128, 8], F32, tag="r", bufs=1, name=f"r{c}")
                    eps = []

                    def _scores(mt):
                        p_sc = pps.tile([128, 512], F32, tag="s", name=f"sc{c}_{mt}")
                        for dk in range(4):
                            nc.tensor.matmul(
                                p_sc[:], aq_sb[:, dk, mt * 128:(mt + 1) * 128],
                                gtb_c[:, :, dk, :], start=(dk == 0), stop=False)
                        for dk in range(4):
                            nc.tensor.matmul(
                                p_sc[:], agp_sb[:, dk, mt * 128:(mt + 1) * 128],
                                gptb_c[:, :, dk, :], start=False, stop=(dk == 3))
                        ep = eB.tile([128, 512], F32R, tag=f"ep{mt}", name=f"ep{c}_{mt}")
                        nc.scalar.activation(out=ep, in_=p_sc[:], func=EXP,
                                             bias=c_sb[:, mt * 2:mt * 2 + 1], scale=1.0)
                        eps.append(ep)

                    def _ov(mt):
                        for nk in range(4):
                            nc.tensor.matmul(
                                ps_ov[nk][:], eps[mt][:, nk * 128:(nk + 1) * 128],
                                v_sb[:, mt, :], start=(mt == 0), stop=(mt == 3))

                    # software-pipelined: scores(mt+1) is emitted before OV(mt)
                    # so the PE never waits on the exp of the tile it just
                    # produced.
                    _scores(0)
                    for mt in range(4):
                        if mt + 1 < 4:
                            _scores(mt + 1)
                        _ov(mt)
                    for nk in range(4):
                        for mt in range(4):
                            nc.tensor.matmul(
                                ps_r[:, nk * 2:nk * 2 + 2],
                                eps[mt][:, nk * 128:(nk + 1) * 128],
                                ones_c[:], start=(mt == 0), stop=(mt == 3))
                    rr_sb = eB.tile([128, 8], F32, tag="rr", name=f"rr{c}")
                    nc.vector.reciprocal(out=rr_sb, in_=ps_r[:])
                    s1_4 = eB.tile([128, 4], F32, tag="s1", name=f"s1{c}")
                    s2_4 = eB.tile([128, 4], F32, tag="s2", name=f"s2{c}")
                    pres = []
                    for nk in range(4):
                        pre = eB.tile([128, 512], F32, tag=f"pre{nk}",
                                      name=f"pre{c}_{nk}")
                        nc.vector.scalar_tensor_tensor(
                            out=pre, in0=ps_ov[nk][:], scalar=rr_sb[:, nk * 2:nk * 2 + 1],
                            in1=gres_c[:, nk, :], op0=MULT, op1=ADD,
                            accum_out=s1_4[:, nk:nk + 1])
                        pres.append(pre)
                        sqj = eB.tile([128, 512], F32, tag=f"sq{nk % 2}",
                                      name=f"sq{c}_{nk}")
                        nc.scalar.activation(out=sqj, in_=pre, func=SQUARE,
                                             accum_out=s2_4[:, nk:nk + 1])
                    # mu = s1/512 ; var = s2/512 - mu^2 ; rstd = exp(-.5*ln(var+eps))
                    mu4 = eB.tile([128, 4], F32, tag="mu", name=f"mu{c}")
                    var4 = eB.tile([128, 4], F32, tag="var", name=f"var{c}")
                    rstd4 = eB.tile([128, 4], F32, tag="rstd", name=f"rstd{c}")
                    nc.vector.tensor_scalar_mul(out=mu4, in0=s1_4, scalar1=1.0 / 512)
                    nc.vector.tensor_mul(out=var4, in0=mu4, in1=mu4)
                    nc.vector.scalar_tensor_tensor(
                        out=var4, in0=s2_4, scalar=1.0 / 512, in1=var4,
                        op0=MULT, op1=SUB)
                    nc.scalar.activation(out=rstd4, in_=var4, func=LN_F, bias=eps_sb)
                    nc.scalar.activation(out=rstd4, in_=rstd4, func=EXP,
                                         scale=nhalf_sb)
                    ob = eB.tile([128, 4, 512], F32, tag="ob", name=f"ob{c}")
                    out_r = out_d.rearrange("(c nk p) d -> c nk p d", p=128, nk=4)
                    for nk in range(4):
                        nc.vector.tensor_scalar(out=ob[:, nk, :], in0=pres[nk],
                                                scalar1=mu4[:, nk:nk + 1],
                                                scalar2=rstd4[:, nk:nk + 1],
                                                op0=SUB, op1=MULT)
                        if not gb_trivial:
                            nc.vector.tensor_mul(out=ob[:, nk, :], in0=ob[:, nk, :],
                                                 in1=gam_bc)
                            nc.vector.tensor_add(out=ob[:, nk, :], in0=ob[:, nk, :],
                                                 in1=bet_bc)
                        # per-nk store on the Act DGE queue: each slab leaves
                        # as soon as its LN finishes (shrinks the tail), and
                        # the sync queue stays a pure prefetch stream.
                        nc.scalar.dma_start(out=out_r[c, nk], in_=ob[:, nk, :])
            pf_ctx.__exit__(None, None, None)
    nc.compile()
    return nc


_CACHE = {}


def _get_nc(n_loc, n_cores, gb_trivial):
    key = (n_loc, n_cores, gb_trivial)
    if key not in _CACHE:
        _CACHE[key] = build(n_loc, n_cores, gb_trivial)
    return _CACHE[key]


def _packT(slab):
    """[n_loc, 512] f32 -> [128, nt, 4, 128] transposed pack: out[p,t,k,j] =
    slab[t*128+j, k*128+p]."""
    n_loc = slab.shape[0]
    return np.ascontiguousarray(
        slab.reshape(n_loc // 128, 128, 4, 128).transpose(3, 0, 2, 1))


def kernel(g, g_p, W, Wq, bq, Wk, bk, Wv, bv, Wgp, bgp, Wkp, bkp, gamma, beta,
           _trace=False):
    g = np.asarray(g, np.float32)
    g_p = np.asarray(g_p, np.float32)
    gamma = np.asarray(gamma, np.float32)
    beta = np.asarray(beta, np.float32)
    gb_trivial = bool(np.all(gamma == 1.0) and np.all(beta == 0.0))
    n = g.shape[0]
    n_loc = n // N_CORES
    nc = _get_nc(n_loc, N_CORES, gb_trivial)

    Ws = np.asarray(W, np.float32).reshape(M, D)
    Wk_ = np.asarray(Wk, np.float32)
    Wv_ = np.asarray(Wv, np.float32)
    shared = {
        # wsT[p,k,m] = Ws[m, k*128+p]
        "wsT": np.ascontiguousarray(Ws.reshape(M, 4, 128).transpose(2, 1, 0)),
        # wkT[p,t,d] = Wk[d, t*128+p]
        "wkT": np.ascontiguousarray(Wk_.T.reshape(4, 128, D).transpose(1, 0, 2)),
        "wvT": np.ascontiguousarray(Wv_.T.reshape(4, 128, D).transpose(1, 0, 2)),
        "wq": np.asarray(Wq, np.float32), "wgp": np.asarray(Wgp, np.float32),
        "wkp": np.asarray(Wkp, np.float32),
        "bq": np.asarray(bq, np.float32), "bk": np.asarray(bk, np.float32),
        "bv": np.asarray(bv, np.float32), "bgp": np.asarray(bgp, np.float32),
        "bkp": np.asarray(bkp, np.float32),
    }
    if not gb_trivial:
        shared["gamma"] = gamma
        shared["beta"] = beta
    in_maps = []
    for cid in range(N_CORES):
        sl = slice(cid * n_loc, (cid + 1) * n_loc)
        g_slab = np.ascontiguousarray(g[sl])
        gp_slab = np.ascontiguousarray(g_p[sl])
        gt32 = _packT(g_slab)
        gpt32 = _packT(gp_slab)
        in_maps.append({
            "gt32": gt32,
            "gp32": gp_slab,
            "gtb": gt32.astype(bfloat16),
            "gptb": gpt32.astype(bfloat16),
            "gres": g_slab,
            **shared,
        })
    res = bass_utils.run_bass_kernel_spmd(
        nc, in_maps, core_ids=list(range(N_CORES)), trace=_trace)
    out = np.concatenate([res.results[cid]["out"] for cid in range(N_CORES)], axis=0)
    if _trace:
        return out, res
    return out


# revision 34
# speedup vs baseline: 1.0092x; 1.0092x over previous
"""Trainium2 Bass kernel for nn_CrossAttention (N=65536 gaussians, M=512 tokens, D=512).

Runs SPMD on 8 NeuronCores; N sharded across cores.

Host marshalling supplies g/g_p in both natural layout (f32) and transposed
packed layout ([128, nt, 4, 128]; f32 for the pooling pass, bf16 for the
attention pass), plus pre-transposed weight packs.  This removes all on-device
PE transposes and their PSUM->SBUF copies.

Per core (n_loc rows):
  Phase A (pool): per 128-row tile, p_s = g @ Ws.T via host-gT stationary;
    et = exp(p_s - C_SHIFT); accumulate P.T = gp.T @ et and l = 1.T @ et in
    PSUM across all tiles.
  AllReduce (bf16) of (P.T || l); weight-prep matmuls (K0T, V, Aq0, Agp0,
    GqT, GgpT, c0, u0) are emitted after phase A so they execute during the
    collective.
  Fixups: kplT = P.T/l;  Aq = SCALE*Wq.T@K.T = Aq0 + GqT@kplT (stored bf16),
    likewise Agp; c = c0 + u0-part (score bias from bq+bgp; zero for the
    given inputs but handled generally).
  Phase B (attention): per 512-row chunk, scoresT = Aq.T@gT + Agp.T@gpT in
    bf16; ep = exp(scoresT + c); OV and rowsum r via PE; LN fused:
    pre = OV*(1/r) + g (scalar_tensor_tensor, accum -> mean),
    sumsq via activation Square accum, rstd = exp(-0.5*ln(var+eps)),
    out = (pre - mu)*rstd [*gamma + beta if non-trivial].
"""
import numpy as np
from ml_dtypes import bfloat16

import concourse.bass as bass
import concourse.tile as tile
from concourse import bacc, bass_isa, mybir, bass_utils

N_CORES = 8
N_FULL = 65536
D = 512
M = 512
SCALE = (D // 8) ** -0.5  # 0.125
LN_EPS = 1e-5
C_SHIFT = 115.0
F32 = mybir.dt.float32
F32R = mybir.dt.float32r
BF16 = mybir.dt.bfloat16
EXP = mybir.ActivationFunctionType.Exp
LN_F = mybir.ActivationFunctionType.Ln
SQUARE = mybir.ActivationFunctionType.Square
MULT = mybir.AluOpType.mult
ADD = mybir.AluOpType.add
SUB = mybir.AluOpType.subtract


def _bcast(ap, parts):
    """Partition-broadcast a [F]-shaped DRAM AP to [parts, F] for DMA."""
    return bass.AP(tensor=ap.tensor, offset=ap.offset, ap=[[0, parts], *ap.ap])


def build(n_loc=N_FULL // N_CORES, n_cores=N_CORES, gb_trivial=True):
    nt = n_loc // 128      # phase-A tiles of 128 rows
    nch = n_loc // 512     # phase-B chunks of 512 rows
    assert n_loc % 512 == 0

    nc = bacc.Bacc("TRN2", target_bir_lowering=False, debug=False, num_devices=n_cores)
    gt32_d = nc.dram_tensor("gt32", [128, nt, 4, 128], F32R, kind="ExternalInput").ap()
    gpb_d = nc.dram_tensor("gpb", [n_loc, D], BF16, kind="ExternalInput").ap()
    gtb_d = nc.dram_tensor("gtb", [128, nt, 4, 128], BF16, kind="ExternalInput").ap()
    gptb_d = nc.dram_tensor("gptb", [128, nt, 4, 128], BF16, kind="ExternalInput").ap()
    gres_d = nc.dram_tensor("gres", [n_loc, D], BF16, kind="ExternalInput").ap()
    wsT_d = nc.dram_tensor("wsT", [128, 4, D], F32R, kind="ExternalInput").ap()
    wkT_d = nc.dram_tensor("wkT", [128, 4, D], F32R, kind="ExternalInput").ap()
    wvT_d = nc.dram_tensor("wvT", [128, 4, D], F32R, kind="ExternalInput").ap()
    wq_d = nc.dram_tensor("wq", [D, D], F32R, kind="ExternalInput").ap()
    wgp_d = nc.dram_tensor("wgp", [D, D], F32R, kind="ExternalInput").ap()
    wkp_d = nc.dram_tensor("wkp", [D, D], F32R, kind="ExternalInput").ap()
    bq_d = nc.dram_tensor("bq", [D], F32, kind="ExternalInput").ap()
    bk_d = nc.dram_tensor("bk", [D], F32, kind="ExternalInput").ap()
    bv_d = nc.dram_tensor("bv", [D], F32, kind="ExternalInput").ap()
    bgp_d = nc.dram_tensor("bgp", [D], F32, kind="ExternalInput").ap()
    bkp_d = nc.dram_tensor("bkp", [D], F32, kind="ExternalInput").ap()
    if not gb_trivial:
        gam_d = nc.dram_tensor("gamma", [D], F32, kind="ExternalInput").ap()
        bet_d = nc.dram_tensor("beta", [D], F32, kind="ExternalInput").ap()
    out_d = nc.dram_tensor("out", [n_loc, D], F32, kind="ExternalOutput").ap()

    with tile.TileContext(nc) as tc:
        with (
            tc.tile_pool(name="wts", bufs=1) as wts,
            tc.tile_pool(name="ps", bufs=3, space="PSUM") as pps,
            tc.tile_pool(name="dram", bufs=1, space="DRAM") as dpool,
        ):
            # ---------- persistent tiles ----------
            # Pin the activation table to set 6 (natural_log_exp_and_others:
            # exp+ln+copy+square) so the table-load pass never alternates
            # between the exp-first and ln-first sets (1.28us per reload).
            nc.scalar.add_instruction(mybir.InstLoadActFuncSet(
                name=nc.get_next_instruction_name(),
                act_func_set_id=6, ins=[], outs=[]))
            ones_f = wts.tile([128, 128], F32)
            nc.vector.memset(ones_f, 1.0)
            ones_m = wts.tile([128, 128], F32R)
            nc.vector.tensor_copy(out=ones_m, in_=ones_f)
            ones_c = wts.tile([128, 2], F32R)
            nc.vector.tensor_copy(out=ones_c, in_=ones_f[:, 0:2])
            negc_sb = wts.tile([128, 1], F32)
            nc.vector.memset(negc_sb, -C_SHIFT)
            eps_sb = wts.tile([128, 1], F32)
            nc.vector.memset(eps_sb, LN_EPS)
            nhalf_sb = wts.tile([128, 1], F32)
            nc.vector.memset(nhalf_sb, -0.5)
            wsT = wts.tile([128, 4, D], F32R)
            for k in range(4):
                nc.sync.dma_start(out=wsT[:, k, :], in_=wsT_d[:, k, :])
            v_sb = wts.tile([128, 4, D], F32R)   # V [m-part, mt, d]
            aq_sb = wts.tile([128, 4, D], BF16)  # SCALE*Wq.T@K.T [d-part, dt, m]
            agp_sb = wts.tile([128, 4, D], BF16)
            c_sb = wts.tile([128, 8], F32)       # score bias c[m] as [m-part, (mt,2)]
            bv_bc = wts.tile([128, D], F32)
            nc.scalar.dma_start(out=bv_bc, in_=_bcast(bv_d, 128))
            if not gb_trivial:
                gam_bc = wts.tile([128, D], F32)
                bet_bc = wts.tile([128, D], F32)
                nc.scalar.dma_start(out=gam_bc, in_=_bcast(gam_d, 128))
                nc.scalar.dma_start(out=bet_bc, in_=_bcast(bet_d, 128))
            # weight/bias tiles persist in wts; their DMAs ride the Act-engine
            # DGE queue and are emitted a few tiles into phase A so the first
            # g-tile loads aren't bandwidth-starved.
            wkT = wts.tile([128, 4, D], F32R)
            wvT = wts.tile([128, 4, D], F32R)
            wq_n = wts.tile([128, 4, D], F32R)
            wgp_n = wts.tile([128, 4, D], F32R)
            wkp_n = wts.tile([128, 4, D], F32R)

            def _weight_dmas():
                nc.scalar.dma_start(out=wkT, in_=wkT_d)
                nc.scalar.dma_start(out=wvT, in_=wvT_d)
                for t_sb, t_d in [(wq_n, wq_d), (wgp_n, wgp_d), (wkp_n, wkp_d)]:
                    nc.scalar.dma_start(
                        out=t_sb, in_=t_d.rearrange("(t p) d -> p t d", p=128))
            bk_sb = wts.tile([128, 4], F32)
            bs_sb = wts.tile([128, 4], F32)
            bq_sb = wts.tile([128, 4], F32)
            bkp_sb = wts.tile([128, 4], F32)
            nc.scalar.dma_start(out=bk_sb, in_=bk_d.rearrange("(t p) -> p t", p=128))
            nc.scalar.dma_start(out=bq_sb, in_=bq_d.rearrange("(t p) -> p t", p=128))
            nc.scalar.dma_start(out=bs_sb, in_=bgp_d.rearrange("(t p) -> p t", p=128))
            nc.scalar.dma_start(out=bkp_sb,
                                in_=bkp_d.rearrange("(t p) -> p t", p=128))

            # warm the PE pstate during the initial DMA wait
            p_w = pps.tile([128, 128], F32, tag="s", name="p_warm")
            for w in range(24):
                nc.tensor.matmul(p_w[:], ones_m[:], ones_m[:],
                                 start=(w == 0), stop=(w == 23))

            # ---------- phase A: pooling partials ----------
            ctxA = tc.tile_pool(name="pAacc", bufs=1, space="PSUM")
            pAacc = ctxA.__enter__()
            ps_p = [pAacc.tile([128, 512], F32, tag=f"psp{i}", bufs=1,
                               name=f"ps_p{i}") for i in range(4)]
            l_acc = wts.tile([128, 512], F32)
            nc.vector.memset(l_acc, 0.0)
            with tc.tile_pool(name="sA", bufs=4) as sA:
                for i in range(nt):
                    if i == 6:
                        _weight_dmas()
                    gT_t = sA.tile([128, 4, 128], F32R, tag="gTA", name=f"gTA{i}")
                    gpn = sA.tile([128, D], BF16, tag="gpnA", name=f"gpnA{i}")
                    nc.sync.dma_start(out=gT_t, in_=gt32_d[:, i, :, :])
                    nc.sync.dma_start(out=gpn, in_=gpb_d[i * 128:(i + 1) * 128, :])
                    p_s = pps.tile([128, 512], F32, tag="s", name=f"psA{i}")
                    for dk in range(4):
                        nc.tensor.matmul(p_s[:], gT_t[:, dk, :], wsT[:, dk, :],
                                         start=(dk == 0), stop=(dk == 3))
                    et = sA.tile([128, 512], F32R, tag="etA", name=f"etA{i}")
                    nc.scalar.activation(out=et, in_=p_s[:], func=EXP,
                                         bias=negc_sb, scale=1.0)
                    for d2t in range(4):
                        nc.tensor.matmul(
                            ps_p[d2t][:], gpn[:, d2t * 128:(d2t + 1) * 128], et[:],
                            start=(i == 0), stop=(i == nt - 1))
                    # l += colsums(et) off the PE: partition-reduce on the Q7
                    # (Pool) engine, accumulate on the (idle) DVE.
                    l_i = sA.tile([128, 512], F32, tag="liA", name=f"liA{i}")
                    nc.gpsimd.partition_all_reduce(
                        l_i, et[:], channels=128, reduce_op=bass_isa.ReduceOp.add)
                    nc.vector.tensor_add(out=l_acc, in0=l_acc, in1=l_i)

            # ---------- all-reduce of (P.T || l), bf16 ----------
            # pf opens BEFORE w0 so its SBUF region does not overlap w0's
            # tiles; its prefetch DMAs (sync queue) can then run during the
            # collective without anti-dependency stalls.
            pf_ctx = tc.tile_pool(name="pf", bufs=4)
            pf = pf_ctx.__enter__()
            pf_tiles = {}

            def _prefetch(c):
                gtb_c = pf.tile([128, 4, 4, 128], BF16, tag="gtb", name=f"gtb{c}")
                gptb_c = pf.tile([128, 4, 4, 128], BF16, tag="gptb", name=f"gptb{c}")
                gres_c = pf.tile([128, 4, 512], BF16, tag="gres", name=f"gres{c}")
                nc.sync.dma_start(out=gtb_c, in_=gtb_d[:, 4 * c:4 * c + 4, :, :])
                nc.sync.dma_start(out=gptb_c, in_=gptb_d[:, 4 * c:4 * c + 4, :, :])
                nc.sync.dma_start(
                    out=gres_c,
                    in_=gres_d[c * 512:(c + 1) * 512, :].rearrange(
                        "(nk p) d -> p nk d", p=128))
                pf_tiles[c] = (gtb_c, gptb_c, gres_c)

            with tc.tile_pool(name="arp", bufs=1) as arp:
                pl_sb = arp.tile([128, 5, 512], BF16)
                for d2t in range(4):
                    if d2t % 2 == 0:
                        nc.vector.tensor_copy(out=pl_sb[:, d2t, :], in_=ps_p[d2t][:])
                    else:
                        nc.scalar.copy(out=pl_sb[:, d2t, :], in_=ps_p[d2t][:])
                nc.vector.tensor_copy(out=pl_sb[:, 4, :], in_=l_acc)
                ctxA.__exit__(None, None, None)
                # ReduceScatter + AllGather = AllReduce, but ~20% cheaper on
                # the interconnect (no duplicate-reduce traffic).
                ar_in = dpool.tile([128, 5 * 512], BF16)
                rs_out = dpool.tile([128, 5 * 512 // n_cores], BF16)
                ar_out = dpool.tile([128, 5 * 512], BF16, addr_space="Shared")
                nc.sync.dma_start(out=ar_in[:],
                                  in_=pl_sb[:].rearrange("p a b -> p (a b)"))
                nc.gpsimd.collective_compute(
                    "ReduceScatter", mybir.AluOpType.add,
                    replica_groups=[list(range(n_cores))],
                    ins=[ar_in.opt()], outs=[rs_out.opt()])
                nc.gpsimd.collective_compute(
                    "AllGather", mybir.AluOpType.bypass,
                    replica_groups=[list(range(n_cores))],
                    ins=[rs_out.opt()], outs=[ar_out.opt()])
                # phase-B prefetch begins immediately; these loads only await
                # free pf buffers, so they overlap the collective.
                for c in range(min(4, nch)):
                    _prefetch(c)

                # ---------- phase 0 weight prep (overlaps the collective) ----
                with tc.tile_pool(name="w0", bufs=1) as w0:
                    nc.vector.tensor_add(out=bs_sb, in0=bs_sb, in1=bq_sb)
                    # K.T = Wk@Ws.T + (bk + bkp) + Wkp@kpool.T -> fold bkp into K0T bias
                    nc.vector.tensor_add(out=bk_sb, in0=bk_sb, in1=bkp_sb)

                    k0T = w0.tile([128, 4, D], F32R)
                    aq0 = w0.tile([128, 4, D], F32)
                    agp0 = w0.tile([128, 4, D], F32)
                    gqT = w0.tile([128, 4, D], F32R)
                    ggpT = w0.tile([128, 4, D], F32R)
                    c0_sb = w0.tile([128, 8], F32)
                    u0c2 = w0.tile([128, 4, 2], F32R)
                    # K0T[d, m] = Wk @ Ws.T + bk'
                    for dt in range(4):
                        p_k = pps.tile([128, 512], F32, tag="s", name=f"pk{dt}")
                        for di in range(4):
                            nc.tensor.matmul(p_k[:], wkT[:, di, dt * 128:(dt + 1) * 128],
                                             wsT[:, di, :], start=(di == 0), stop=(di == 3))
                        nc.vector.tensor_scalar_add(out=k0T[:, dt, :], in0=p_k[:],
                                                    scalar1=bk_sb[:, dt:dt + 1])
                    # V[m, d] = Ws @ Wv.T + bv
                    for mt in range(4):
                        p_v = pps.tile([128, 512], F32, tag="s", name=f"pv{mt}")
                        for di in range(4):
                            nc.tensor.matmul(p_v[:], wsT[:, di, mt * 128:(mt + 1) * 128],
                                             wvT[:, di, :], start=(di == 0), stop=(di == 3))
                        nc.vector.tensor_add(out=v_sb[:, mt, :], in0=p_v[:], in1=bv_bc)
                    # Aq0 = SCALE*Wq.T@K0T ; Agp0 likewise
                    for w_nat, dst in [(wq_n, aq0), (wgp_n, agp0)]:
                        for dt in range(4):
                            p_a = pps.tile([128, 512], F32, tag="s",
                                           name=f"pa_{dst.tensor.name}_{dt}")
                            for di in range(4):
                                nc.tensor.matmul(
                                    p_a[:], w_nat[:, di, dt * 128:(dt + 1) * 128],
                                    k0T[:, di, :], start=(di == 0), stop=(di == 3))
                            nc.scalar.mul(out=dst[:, dt, :], in_=p_a[:], mul=SCALE)
                    # GqT = SCALE*(Wkp.T @ Wq) ; GgpT likewise
                    for w_nat, dst in [(wq_n, gqT), (wgp_n, ggpT)]:
                        for dt in range(4):
                            p_gq = pps.tile([128, 512], F32, tag="s",
                                            name=f"pg_{dst.tensor.name}_{dt}")
                            for di in range(4):
                                nc.tensor.matmul(
                                    p_gq[:], wkp_n[:, di, dt * 128:(dt + 1) * 128],
                                    w_nat[:, di, :], start=(di == 0), stop=(di == 3))
                            nc.scalar.mul(out=dst[:, dt, :], in_=p_gq[:], mul=SCALE)
                    # c0[m] = SCALE*(bq+bgp)@K0T ; u0 = SCALE*Wkp.T@(bq+bgp)
                    bsr2 = w0.tile([128, 4, 2], F32R)
                    nc.vector.tensor_copy(out=bsr2[:, :, 0], in_=bs_sb)
                    nc.vector.tensor_copy(out=bsr2[:, :, 1], in_=bs_sb)
                    ctx0 = tc.tile_pool(name="p0acc", bufs=1, space="PSUM")
                    p0acc = ctx0.__enter__()
                    p_c0 = p0acc.tile([128, 8], F32, tag="pc0", bufs=1, name="p_c0")
                    p_u0 = p0acc.tile([128, 8], F32, tag="pu0", bufs=1, name="p_u0")
                    for mt in range(4):
                        for di in range(4):
                            nc.tensor.matmul(
                                p_c0[:, mt * 2:mt * 2 + 2],
                                k0T[:, di, mt * 128:(mt + 1) * 128],
                                bsr2[:, di, :], start=(di == 0), stop=(di == 3))
                            nc.tensor.matmul(
                                p_u0[:, mt * 2:mt * 2 + 2],
                                wkp_n[:, di, mt * 128:(mt + 1) * 128],
                                bsr2[:, di, :], start=(di == 0), stop=(di == 3))
                    nc.scalar.mul(out=c0_sb, in_=p_c0[:], mul=SCALE)
                    nc.scalar.mul(out=u0c2.rearrange("p a b -> p (a b)"), in_=p_u0[:],
                                  mul=SCALE)
                    ctx0.__exit__(None, None, None)

                    # ---------- post-collective fixups ----------
                    # plr rides the Act-engine DGE queue (idle here), keeping
                    # the sync queue free for phase-B prefetch.
                    plr_sb = w0.tile([128, 5, 512], BF16)
                    nc.scalar.dma_start(out=plr_sb,
                                        in_=ar_out[:].rearrange("p (a b) -> p a b", a=5))
                    lr_sb = w0.tile([128, 512], F32)
                    nc.vector.reciprocal(out=lr_sb, in_=plr_sb[:, 4, :])
                    kplT = w0.tile([128, 4, D], F32R)
                    for dint in range(4):
                        nc.vector.tensor_mul(out=kplT[:, dint, :],
                                             in0=plr_sb[:, dint, :], in1=lr_sb)
                    for gT_w, base, dst in [(gqT, aq0, aq_sb), (ggpT, agp0, agp_sb)]:
                        for dt in range(4):
                            p_aq = pps.tile([128, 512], F32, tag="s",
                                            name=f"paq_{dst.tensor.name}_{dt}")
                            for di in range(4):
                                nc.tensor.matmul(
                                    p_aq[:], gT_w[:, di, dt * 128:(dt + 1) * 128],
                                    kplT[:, di, :], start=(di == 0), stop=(di == 3))
                            nc.vector.tensor_add(out=dst[:, dt, :], in0=base[:, dt, :],
                                                 in1=p_aq[:])
                    ctxP = tc.tile_pool(name="pPacc", bufs=1, space="PSUM")
                    pPacc = ctxP.__enter__()
                    p_cp = pPacc.tile([128, 8], F32, tag="pcp", bufs=1, name="p_cp")
                    for mt in range(4):
                        for di in range(4):
                            nc.tensor.matmul(
                                p_cp[:, mt * 2:mt * 2 + 2],
                                kplT[:, di, mt * 128:(mt + 1) * 128],
                                u0c2[:, di, :], start=(di == 0), stop=(di == 3))
                    nc.vector.tensor_add(out=c_sb, in0=c0_sb, in1=p_cp[:])
                    ctxP.__exit__(None, None, None)

            # ---------- phase B: attention ----------
            with (tc.tile_pool(name="eB", bufs=2) as eB,
                  tc.tile_pool(name="pBacc", bufs=1, space="PSUM") as pBacc):
                for c in range(nch):
                    if c + 4 < nch:
                        _prefetch(c + 4)
                    gtb_c, gptb_c, gres_c = pf_tiles.pop(c)
                    ps_ov = [pBacc.tile([128, 512], F32, tag=f"ov{k}", bufs=1,
                                        name=f"ov{c}_{k}") for k in range(4)]
                    ps_r = pBacc.tile([128, 8], F32, tag="r", bufs=1, name=f"r{c}")
                    eps = []

                    def _scores(mt):
                        p_sc = pps.tile([128, 512], F32, tag="s", name=f"sc{c}_{mt}")
                        for dk in range(4):
                            nc.tensor.matmul(
                                p_sc[:], aq_sb[:, dk, mt * 128:(mt + 1) * 128],
                                gtb_c[:, :, dk, :], start=(dk == 0), stop=False)
                        for dk in range(4):
                            nc.tensor.matmul(
                                p_sc[:], agp_sb[:, dk, mt * 128:(mt + 1) * 128],
                                gptb_c[:, :, dk, :], start=False, stop=(dk == 3))
                        ep = eB.tile([128, 512], F32R, tag=f"ep{mt}", name=f"ep{c}_{mt}")
                        nc.scalar.activation(out=ep, in_=p_sc[:], func=EXP,
                                             bias=c_sb[:, mt * 2:mt * 2 + 1], scale=1.0)
                        eps.append(ep)

                    def _ov(mt):
                        for nk in range(4):
                            nc.tensor.matmul(
                                ps_ov[nk][:], eps[mt][:, nk * 128:(nk + 1) * 128],
                                v_sb[:, mt, :], start=(mt == 0), stop=(mt == 3))

                    # software-pipelined: scores(mt+1) is emitted before OV(mt)
                    # so the PE never waits on the exp of the tile it just
                    # produced.
                    _scores(0)
                    for mt in range(4):
                        if mt + 1 < 4:
                            _scores(mt + 1)
                        _ov(mt)
                    for nk in range(4):
                        for mt in range(4):
                            nc.tensor.matmul(
                                ps_r[:, nk * 2:nk * 2 + 2],
                                eps[mt][:, nk * 128:(nk + 1) * 128],
                                ones_c[:], start=(mt == 0), stop=(mt == 3))
                    rr_sb = eB.tile([128, 8], F32, tag="rr", name=f"rr{c}")
                    nc.vector.reciprocal(out=rr_sb, in_=ps_r[:])
                    s1_4 = eB.tile([128, 4], F32, tag="s1", name=f"s1{c}")
                    s2_4 = eB.tile([128, 4], F32, tag="s2", name=f"s2{c}")
                    pres = []
                    for nk in range(4):
                        pre = eB.tile([128, 512], F32, tag=f"pre{nk}",
                                      name=f"pre{c}_{nk}")
                        nc.vector.scalar_tensor_tensor(
                            out=pre, in0=ps_ov[nk][:], scalar=rr_sb[:, nk * 2:nk * 2 + 1],
                            in1=gres_c[:, nk, :], op0=MULT, op1=ADD,
                            accum_out=s1_4[:, nk:nk + 1])
                        pres.append(pre)
                        sqj = eB.tile([128, 512], F32, tag=f"sq{nk % 2}",
                                      name=f"sq{c}_{nk}")
                        nc.scalar.activation(out=sqj, in_=pre, func=SQUARE,
                                             accum_out=s2_4[:, nk:nk + 1])
                    # mu = s1/512 ; var = s2/512 - mu^2 ; rstd = exp(-.5*ln(var+eps))
                    mu4 = eB.tile([128, 4], F32, tag="mu", name=f"mu{c}")
                    var4 = eB.tile([128, 4], F32, tag="var", name=f"var{c}")
                    rstd4 = eB.tile([128, 4], F32, tag="rstd", name=f"rstd{c}")
                    nc.vector.tensor_scalar_mul(out=mu4, in0=s1_4, scalar1=1.0 / 512)
                    nc.vector.tensor_mul(out=var4, in0=mu4, in1=mu4)
                    nc.vector.scalar_tensor_tensor(
                        out=var4, in0=s2_4, scalar=1.0 / 512, in1=var4,
                        op0=MULT, op1=SUB)
                    nc.scalar.activation(out=rstd4, in_=var4, func=LN_F, bias=eps_sb)
                    nc.scalar.activation(out=rstd4, in_=rstd4, func=EXP,
                                         scale=nhalf_sb)
                    ob = eB.tile([128, 4, 512], F32, tag="ob", name=f"ob{c}")
                    out_r = out_d.rearrange("(c nk p) d -> c nk p d", p=128, nk=4)
                    for nk in range(4):
                        nc.vector.tensor_scalar(out=ob[:, nk, :], in0=pres[nk],
                                                scalar1=mu4[:, nk:nk + 1],
                                                scalar2=rstd4[:, nk:nk + 1],
                                                op0=SUB, op1=MULT)
                        if not gb_trivial:
                            nc.vector.tensor_mul(out=ob[:, nk, :], in0=ob[:, nk, :],
                                                 in1=gam_bc)
                            nc.vector.tensor_add(out=ob[:, nk, :], in0=ob[:, nk, :],
                                                 in1=bet_bc)
                        # per-nk store on the Act DGE queue: each slab leaves
                        # as soon as its LN finishes (shrinks the tail), and
                        # the sync queue stays a pure prefetch stream.
                        nc.scalar.dma_start(out=out_r[c, nk], in_=ob[:, nk, :])
            pf_ctx.__exit__(None, None, None)
    nc.compile()
    return nc


_CACHE = {}


def _get_nc(n_loc, n_cores, gb_trivial):
    key = (n_loc, n_cores, gb_trivial)
    if key not in _CACHE:
        _CACHE[key] = build(n_loc, n_cores, gb_trivial)
    return _CACHE[key]


def _packT(slab):
    """[n_loc, 512] f32 -> [128, nt, 4, 128] transposed pack: out[p,t,k,j] =
    slab[t*128+j, k*128+p]."""
    n_loc = slab.shape[0]
    return np.ascontiguousarray(
        slab.reshape(n_loc // 128, 128, 4, 128).transpose(3, 0, 2, 1))


def kernel(g, g_p, W, Wq, bq, Wk, bk, Wv, bv, Wgp, bgp, Wkp, bkp, gamma, beta,
           _trace=False):
    g = np.asarray(g, np.float32)
    g_p = np.asarray(g_p, np.float32)
    gamma = np.asarray(gamma, np.float32)
    beta = np.asarray(beta, np.float32)
    gb_trivial = bool(np.all(gamma == 1.0) and np.all(beta == 0.0))
    n = g.shape[0]
    n_loc = n // N_CORES
    nc = _get_nc(n_loc, N_CORES, gb_trivial)

    Ws = np.asarray(W, np.float32).reshape(M, D)
    Wk_ = np.asarray(Wk, np.float32)
    Wv_ = np.asarray(Wv, np.float32)
    shared = {
        # wsT[p,k,m] = Ws[m, k*128+p]
        "wsT": np.ascontiguousarray(Ws.reshape(M, 4, 128).transpose(2, 1, 0)),
        # wkT[p,t,d] = Wk[d, t*128+p]
        "wkT": np.ascontiguousarray(Wk_.T.reshape(4, 128, D).transpose(1, 0, 2)),
        "wvT": np.ascontiguousarray(Wv_.T.reshape(4, 128, D).transpose(1, 0, 2)),
        "wq": np.asarray(Wq, np.float32), "wgp": np.asarray(Wgp, np.float32),
        "wkp": np.asarray(Wkp, np.float32),
        "bq": np.asarray(bq, np.float32), "bk": np.asarray(bk, np.float32),
        "bv": np.asarray(bv, np.float32), "bgp": np.asarray(bgp, np.float32),
        "bkp": np.asarray(bkp, np.float32),
    }
    if not gb_trivial:
        shared["gamma"] = gamma
        shared["beta"] = beta
    in_maps = []
    for cid in range(N_CORES):
        sl = slice(cid * n_loc, (cid + 1) * n_loc)
        g_slab = np.ascontiguousarray(g[sl])
        gp_slab = np.ascontiguousarray(g_p[sl])
        gt32 = _packT(g_slab)
        gpt32 = _packT(gp_slab)
        in_maps.append({
            "gt32": gt32,
            "gpb": gp_slab.astype(bfloat16),
            "gtb": gt32.astype(bfloat16),
            "gptb": gpt32.astype(bfloat16),
            "gres": g_slab.astype(bfloat16),
            **shared,
        })
    res = bass_utils.run_bass_kernel_spmd(
        nc, in_maps, core_ids=list(range(N_CORES)), trace=_trace)
    out = np.concatenate([res.results[cid]["out"] for cid in range(N_CORES)], axis=0)
    if _trace:
        return out, res
    return out


# revision 35
# speedup vs baseline: 1.0092x; 1.0000x over previous
"""Trainium2 Bass kernel for nn_CrossAttention (N=65536 gaussians, M=512 tokens, D=512).

Runs SPMD on 8 NeuronCores; N sharded across cores.

Host marshalling supplies g/g_p in both natural layout (f32) and transposed
packed layout ([128, nt, 4, 128]; f32 for the pooling pass, bf16 for the
attention pass), plus pre-transposed weight packs.  This removes all on-device
PE transposes and their PSUM->SBUF copies.

Per core (n_loc rows):
  Phase A (pool): per 128-row tile, p_s = g @ Ws.T via host-gT stationary;
    et = exp(p_s - C_SHIFT); accumulate P.T = gp.T @ et and l = 1.T @ et in
    PSUM across all tiles.
  AllReduce (bf16) of (P.T || l); weight-prep matmuls (K0T, V, Aq0, Agp0,
    GqT, GgpT, c0, u0) are emitted after phase A so they execute during the
    collective.
  Fixups: kplT = P.T/l;  Aq = SCALE*Wq.T@K.T = Aq0 + GqT@kplT (stored bf16),
    likewise Agp; c = c0 + u0-part (score bias from bq+bgp; zero for the
    given inputs but handled generally).
  Phase B (attention): per 512-row chunk, scoresT = Aq.T@gT + Agp.T@gpT in
    bf16; ep = exp(scoresT + c); OV and rowsum r via PE; LN fused:
    pre = OV*(1/r) + g (scalar_tensor_tensor, accum -> mean),
    sumsq via activation Square accum, rstd = exp(-0.5*ln(var+eps)),
    out = (pre - mu)*rstd [*gamma + beta if non-trivial].
"""
import numpy as np
from ml_dtypes import bfloat16

import concourse.bass as bass
import concourse.tile as tile
from concourse import bacc, bass_isa, mybir, bass_utils

N_CORES = 8
N_FULL = 65536
D = 512
M = 512
SCALE = (D // 8) ** -0.5  # 0.125
LN_EPS = 1e-5
C_SHIFT = 115.0
F32 = mybir.dt.float32
F32R = mybir.dt.float32r
BF16 = mybir.dt.bfloat16
EXP = mybir.ActivationFunctionType.Exp
LN_F = mybir.ActivationFunctionType.Ln
SQUARE = mybir.ActivationFunctionType.Square
MULT = mybir.AluOpType.mult
ADD = mybir.AluOpType.add
SUB = mybir.AluOpType.subtract


def _bcast(ap, parts):
    """Partition-broadcast a [F]-shaped DRAM AP to [parts, F] for DMA."""
    return bass.AP(tensor=ap.tensor, offset=ap.offset, ap=[[0, parts], *ap.ap])


def build(n_loc=N_FULL // N_CORES, n_cores=N_CORES, gb_trivial=True):
    nt = n_loc // 128      # phase-A tiles of 128 rows
    nch = n_loc // 512     # phase-B chunks of 512 rows
    assert n_loc % 512 == 0

    nc = bacc.Bacc("TRN2", target_bir_lowering=False, debug=False, num_devices=n_cores)
    gt32_d = nc.dram_tensor("gt32", [128, nt, 4, 128], F32R, kind="ExternalInput").ap()
    gpb_d = nc.dram_tensor("gpb", [n_loc, D], BF16, kind="ExternalInput").ap()
    gtb_d = nc.dram_tensor("gtb", [128, nt, 4, 128], BF16, kind="ExternalInput").ap()
    gptb_d = nc.dram_tensor("gptb", [128, nt, 4, 128], BF16, kind="ExternalInput").ap()
    gres_d = nc.dram_tensor("gres", [n_loc, D], BF16, kind="ExternalInput").ap()
    wsT_d = nc.dram_tensor("wsT", [128, 4, D], F32R, kind="ExternalInput").ap()
    wkT_d = nc.dram_tensor("wkT", [128, 4, D], F32R, kind="ExternalInput").ap()
    wvT_d = nc.dram_tensor("wvT", [128, 4, D], F32R, kind="ExternalInput").ap()
    wq_d = nc.dram_tensor("wq", [D, D], F32R, kind="ExternalInput").ap()
    wgp_d = nc.dram_tensor("wgp", [D, D], F32R, kind="ExternalInput").ap()
    wkp_d = nc.dram_tensor("wkp", [D, D], F32R, kind="ExternalInput").ap()
    bq_d = nc.dram_tensor("bq", [D], F32, kind="ExternalInput").ap()
    bk_d = nc.dram_tensor("bk", [D], F32, kind="ExternalInput").ap()
    bv_d = nc.dram_tensor("bv", [D], F32, kind="ExternalInput").ap()
    bgp_d = nc.dram_tensor("bgp", [D], F32, kind="ExternalInput").ap()
    bkp_d = nc.dram_tensor("bkp", [D], F32, kind="ExternalInput").ap()
    if not gb_trivial:
        gam_d = nc.dram_tensor("gamma", [D], F32, kind="ExternalInput").ap()
        bet_d = nc.dram_tensor("beta", [D], F32, kind="ExternalInput").ap()
    out_d = nc.dram_tensor("out", [n_loc, D], F32, kind="ExternalOutput").ap()

    with tile.TileContext(nc) as tc:
        with (
            tc.tile_pool(name="wts", bufs=1) as wts,
            tc.tile_pool(name="ps", bufs=3, space="PSUM") as pps,
            tc.tile_pool(name="dram", bufs=1, space="DRAM") as dpool,
        ):
            # ---------- persistent tiles ----------
            # Pin the activation table to set 6 (natural_log_exp_and_others:
            # exp+ln+copy+square) so the table-load pass never alternates
            # between the exp-first and ln-first sets (1.28us per reload).
            nc.scalar.add_instruction(mybir.InstLoadActFuncSet(
                name=nc.get_next_instruction_name(),
                act_func_set_id=6, ins=[], outs=[]))
            ones_f = wts.tile([128, 128], F32)
            nc.vector.memset(ones_f, 1.0)
            ones_m = wts.tile([128, 128], F32R)
            nc.vector.tensor_copy(out=ones_m, in_=ones_f)
            ones_c = wts.tile([128, 2], F32R)
            nc.vector.tensor_copy(out=ones_c, in_=ones_f[:, 0:2])
            negc_sb = wts.tile([128, 1], F32)
            nc.vector.memset(negc_sb, -C_SHIFT)
            eps_sb = wts.tile([128, 1], F32)
            nc.vector.memset(eps_sb, LN_EPS)
            nhalf_sb = wts.tile([128, 1], F32)
            nc.vector.memset(nhalf_sb, -0.5)
            wsT = wts.tile([128, 4, D], F32R)
            for k in range(4):
                nc.sync.dma_start(out=wsT[:, k, :], in_=wsT_d[:, k, :])
            v_sb = wts.tile([128, 4, D], F32R)   # V [m-part, mt, d]
            aq_sb = wts.tile([128, 4, D], BF16)  # SCALE*Wq.T@K.T [d-part, dt, m]
            agp_sb = wts.tile([128, 4, D], BF16)
            c_sb = wts.tile([128, 8], F32)       # score bias c[m] as [m-part, (mt,2)]
            bv_bc = wts.tile([128, D], F32)
            nc.scalar.dma_start(out=bv_bc, in_=_bcast(bv_d, 128))
            if not gb_trivial:
                gam_bc = wts.tile([128, D], F32)
                bet_bc = wts.tile([128, D], F32)
                nc.scalar.dma_start(out=gam_bc, in_=_bcast(gam_d, 128))
                nc.scalar.dma_start(out=bet_bc, in_=_bcast(bet_d, 128))
            # weight/bias tiles persist in wts; their DMAs ride the Act-engine
            # DGE queue and are emitted a few tiles into phase A so the first
            # g-tile loads aren't bandwidth-starved.
            wkT = wts.tile([128, 4, D], F32R)
            wvT = wts.tile([128, 4, D], F32R)
            wq_n = wts.tile([128, 4, D], F32R)
            wgp_n = wts.tile([128, 4, D], F32R)
            wkp_n = wts.tile([128, 4, D], F32R)

            def _weight_dmas():
                nc.scalar.dma_start(out=wkT, in_=wkT_d)
                nc.scalar.dma_start(out=wvT, in_=wvT_d)
                for t_sb, t_d in [(wq_n, wq_d), (wgp_n, wgp_d), (wkp_n, wkp_d)]:
                    nc.scalar.dma_start(
                        out=t_sb, in_=t_d.rearrange("(t p) d -> p t d", p=128))
            bk_sb = wts.tile([128, 4], F32)
            bs_sb = wts.tile([128, 4], F32)
            bq_sb = wts.tile([128, 4], F32)
            bkp_sb = wts.tile([128, 4], F32)
            nc.scalar.dma_start(out=bk_sb, in_=bk_d.rearrange("(t p) -> p t", p=128))
            nc.scalar.dma_start(out=bq_sb, in_=bq_d.rearrange("(t p) -> p t", p=128))
            nc.scalar.dma_start(out=bs_sb, in_=bgp_d.rearrange("(t p) -> p t", p=128))
            nc.scalar.dma_start(out=bkp_sb,
                                in_=bkp_d.rearrange("(t p) -> p t", p=128))

            # warm the PE pstate during the initial DMA wait
            p_w = pps.tile([128, 128], F32, tag="s", name="p_warm")
            for w in range(24):
                nc.tensor.matmul(p_w[:], ones_m[:], ones_m[:],
                                 start=(w == 0), stop=(w == 23))

            # ---------- phase A: pooling partials ----------
            ctxA = tc.tile_pool(name="pAacc", bufs=1, space="PSUM")
            pAacc = ctxA.__enter__()
            ps_p = [pAacc.tile([128, 512], F32, tag=f"psp{i}", bufs=1,
                               name=f"ps_p{i}") for i in range(4)]
            l_acc = wts.tile([128, 512], F32)
            nc.vector.memset(l_acc, 0.0)
            with tc.tile_pool(name="sA", bufs=4) as sA:
                for i in range(nt):
                    if i == 6:
                        _weight_dmas()
                    gT_t = sA.tile([128, 4, 128], F32R, tag="gTA", name=f"gTA{i}")
                    gpn = sA.tile([128, D], BF16, tag="gpnA", name=f"gpnA{i}")
                    nc.sync.dma_start(out=gT_t, in_=gt32_d[:, i, :, :])
                    nc.sync.dma_start(out=gpn, in_=gpb_d[i * 128:(i + 1) * 128, :])
                    p_s = pps.tile([128, 512], F32, tag="s", name=f"psA{i}")
                    for dk in range(4):
                        nc.tensor.matmul(p_s[:], gT_t[:, dk, :], wsT[:, dk, :],
                                         start=(dk == 0), stop=(dk == 3))
                    et = sA.tile([128, 512], BF16, tag="etA", name=f"etA{i}")
                    nc.scalar.activation(out=et, in_=p_s[:], func=EXP,
                                         bias=negc_sb, scale=1.0)
                    for d2t in range(4):
                        nc.tensor.matmul(
                            ps_p[d2t][:], gpn[:, d2t * 128:(d2t + 1) * 128], et[:],
                            start=(i == 0), stop=(i == nt - 1))
                    # l += colsums(et) off the PE: partition-reduce on the Q7
                    # (Pool) engine, accumulate on the (idle) DVE.
                    l_i = sA.tile([128, 512], F32, tag="liA", name=f"liA{i}")
                    nc.gpsimd.partition_all_reduce(
                        l_i, et[:], channels=128, reduce_op=bass_isa.ReduceOp.add)
                    nc.vector.tensor_add(out=l_acc, in0=l_acc, in1=l_i)

            # ---------- all-reduce of (P.T || l), bf16 ----------
            # pf opens BEFORE w0 so its SBUF region does not overlap w0's
            # tiles; its prefetch DMAs (sync queue) can then run during the
            # collective without anti-dependency stalls.
            pf_ctx = tc.tile_pool(name="pf", bufs=4)
            pf = pf_ctx.__enter__()
            pf_tiles = {}

            def _prefetch(c):
                gtb_c = pf.tile([128, 4, 4, 128], BF16, tag="gtb", name=f"gtb{c}")
                gptb_c = pf.tile([128, 4, 4, 128], BF16, tag="gptb", name=f"gptb{c}")
                gres_c = pf.tile([128, 4, 512], BF16, tag="gres", name=f"gres{c}")
                nc.sync.dma_start(out=gtb_c, in_=gtb_d[:, 4 * c:4 * c + 4, :, :])
                nc.sync.dma_start(out=gptb_c, in_=gptb_d[:, 4 * c:4 * c + 4, :, :])
                nc.sync.dma_start(
                    out=gres_c,
                    in_=gres_d[c * 512:(c + 1) * 512, :].rearrange(
                        "(nk p) d -> p nk d", p=128))
                pf_tiles[c] = (gtb_c, gptb_c, gres_c)

            with tc.tile_pool(name="arp", bufs=1) as arp:
                pl_sb = arp.tile([128, 5, 512], BF16)
                for d2t in range(4):
                    if d2t % 2 == 0:
                        nc.vector.tensor_copy(out=pl_sb[:, d2t, :], in_=ps_p[d2t][:])
                    else:
                        nc.scalar.copy(out=pl_sb[:, d2t, :], in_=ps_p[d2t][:])
                nc.vector.tensor_copy(out=pl_sb[:, 4, :], in_=l_acc)
                ctxA.__exit__(None, None, None)
                # ReduceScatter + AllGather = AllReduce, but ~20% cheaper on
                # the interconnect (no duplicate-reduce traffic).
                ar_in = dpool.tile([128, 5 * 512], BF16)
                rs_out = dpool.tile([128, 5 * 512 // n_cores], BF16)
                ar_out = dpool.tile([128, 5 * 512], BF16, addr_space="Shared")
                nc.sync.dma_start(out=ar_in[:],
                                  in_=pl_sb[:].rearrange("p a b -> p (a b)"))
                nc.gpsimd.collective_compute(
                    "ReduceScatter", mybir.AluOpType.add,
                    replica_groups=[list(range(n_cores))],
                    ins=[ar_in.opt()], outs=[rs_out.opt()])
                nc.gpsimd.collective_compute(
                    "AllGather", mybir.AluOpType.bypass,
                    replica_groups=[list(range(n_cores))],
                    ins=[rs_out.opt()], outs=[ar_out.opt()])
                # phase-B prefetch begins immediately; these loads only await
                # free pf buffers, so they overlap the collective.
                for c in range(min(4, nch)):
                    _prefetch(c)

                # ---------- phase 0 weight prep (overlaps the collective) ----
                with tc.tile_pool(name="w0", bufs=1) as w0:
                    nc.vector.tensor_add(out=bs_sb, in0=bs_sb, in1=bq_sb)
                    # K.T = Wk@Ws.T + (bk + bkp) + Wkp@kpool.T -> fold bkp into K0T bias
                    nc.vector.tensor_add(out=bk_sb, in0=bk_sb, in1=bkp_sb)

                    k0T = w0.tile([128, 4, D], F32R)
                    aq0 = w0.tile([128, 4, D], F32)
                    agp0 = w0.tile([128, 4, D], F32)
                    gqT = w0.tile([128, 4, D], F32R)
                    ggpT = w0.tile([128, 4, D], F32R)
                    c0_sb = w0.tile([128, 8], F32)
                    u0c2 = w0.tile([128, 4, 2], F32R)
                    # K0T[d, m] = Wk @ Ws.T + bk'
                    for dt in range(4):
                        p_k = pps.tile([128, 512], F32, tag="s", name=f"pk{dt}")
                        for di in range(4):
                            nc.tensor.matmul(p_k[:], wkT[:, di, dt * 128:(dt + 1) * 128],
                                             wsT[:, di, :], start=(di == 0), stop=(di == 3))
                        nc.vector.tensor_scalar_add(out=k0T[:, dt, :], in0=p_k[:],
                                                    scalar1=bk_sb[:, dt:dt + 1])
                    # V[m, d] = Ws @ Wv.T + bv
                    for mt in range(4):
                        p_v = pps.tile([128, 512], F32, tag="s", name=f"pv{mt}")
                        for di in range(4):
                            nc.tensor.matmul(p_v[:], wsT[:, di, mt * 128:(mt + 1) * 128],
                                             wvT[:, di, :], start=(di == 0), stop=(di == 3))
                        nc.vector.tensor_add(out=v_sb[:, mt, :], in0=p_v[:], in1=bv_bc)
                    # Aq0 = SCALE*Wq.T@K0T ; Agp0 likewise
                    for w_nat, dst in [(wq_n, aq0), (wgp_n, agp0)]:
                        for dt in range(4):
                            p_a = pps.tile([128, 512], F32, tag="s",
                                           name=f"pa_{dst.tensor.name}_{dt}")
                            for di in range(4):
                                nc.tensor.matmul(
                                    p_a[:], w_nat[:, di, dt * 128:(dt + 1) * 128],
                                    k0T[:, di, :], start=(di == 0), stop=(di == 3))
                            nc.scalar.mul(out=dst[:, dt, :], in_=p_a[:], mul=SCALE)
                    # GqT = SCALE*(Wkp.T @ Wq) ; GgpT likewise
                    for w_nat, dst in [(wq_n, gqT), (wgp_n, ggpT)]:
                        for dt in range(4):
                            p_gq = pps.tile([128, 512], F32, tag="s",
                                            name=f"pg_{dst.tensor.name}_{dt}")
                            for di in range(4):
                                nc.tensor.matmul(
                                    p_gq[:], wkp_n[:, di, dt * 128:(dt + 1) * 128],
                                    w_nat[:, di, :], start=(di == 0), stop=(di == 3))
                            nc.scalar.mul(out=dst[:, dt, :], in_=p_gq[:], mul=SCALE)
                    # c0[m] = SCALE*(bq+bgp)@K0T ; u0 = SCALE*Wkp.T@(bq+bgp)
                    bsr2 = w0.tile([128, 4, 2], F32R)
                    nc.vector.tensor_copy(out=bsr2[:, :, 0], in_=bs_sb)
                    nc.vector.tensor_copy(out=bsr2[:, :, 1], in_=bs_sb)
                    ctx0 = tc.tile_pool(name="p0acc", bufs=1, space="PSUM")
                    p0acc = ctx0.__enter__()
                    p_c0 = p0acc.tile([128, 8], F32, tag="pc0", bufs=1, name="p_c0")
                    p_u0 = p0acc.tile([128, 8], F32, tag="pu0", bufs=1, name="p_u0")
                    for mt in range(4):
                        for di in range(4):
                            nc.tensor.matmul(
                                p_c0[:, mt * 2:mt * 2 + 2],
                                k0T[:, di, mt * 128:(mt + 1) * 128],
                                bsr2[:, di, :], start=(di == 0), stop=(di == 3))
                            nc.tensor.matmul(
                                p_u0[:, mt * 2:mt * 2 + 2],
                                wkp_n[:, di, mt * 128:(mt + 1) * 128],
                                bsr2[:, di, :], start=(di == 0), stop=(di == 3))
                    nc.scalar.mul(out=c0_sb, in_=p_c0[:], mul=SCALE)
                    nc.scalar.mul(out=u0c2.rearrange("p a b -> p (a b)"), in_=p_u0[:],
                                  mul=SCALE)
                    ctx0.__exit__(None, None, None)

                    # ---------- post-collective fixups ----------
                    # plr rides the Act-engine DGE queue (idle here), keeping
                    # the sync queue free for phase-B prefetch.
                    plr_sb = w0.tile([128, 5, 512], BF16)
                    nc.scalar.dma_start(out=plr_sb,
                                        in_=ar_out[:].rearrange("p (a b) -> p a b", a=5))
                    lr_sb = w0.tile([128, 512], F32)
                    nc.vector.reciprocal(out=lr_sb, in_=plr_sb[:, 4, :])
                    kplT = w0.tile([128, 4, D], F32R)
                    for dint in range(4):
                        nc.vector.tensor_mul(out=kplT[:, dint, :],
                                             in0=plr_sb[:, dint, :], in1=lr_sb)
                    for gT_w, base, dst in [(gqT, aq0, aq_sb), (ggpT, agp0, agp_sb)]:
                        for dt in range(4):
                            p_aq = pps.tile([128, 512], F32, tag="s",
                                            name=f"paq_{dst.tensor.name}_{dt}")
                            for di in range(4):
                                nc.tensor.matmul(
                                    p_aq[:], gT_w[:, di, dt * 128:(dt + 1) * 128],
                                    kplT[:, di, :], start=(di == 0), stop=(di == 3))
                            nc.vector.tensor_add(out=dst[:, dt, :], in0=base[:, dt, :],
                                                 in1=p_aq[:])
                    ctxP = tc.tile_pool(name="pPacc", bufs=1, space="PSUM")
                    pPacc = ctxP.__enter__()
                    p_cp = pPacc.tile([128, 8], F32, tag="pcp", bufs=1, name="p_cp")
                    for mt in range(4):
                        for di in range(4):
                            nc.tensor.matmul(
                                p_cp[:, mt * 2:mt * 2 + 2],
                                kplT[:, di, mt * 128:(mt + 1) * 128],
                                u0c2[:, di, :], start=(di == 0), stop=(di == 3))
                    nc.vector.tensor_add(out=c_sb, in0=c0_sb, in1=p_cp[:])
                    ctxP.__exit__(None, None, None)

            # ---------- phase B: attention ----------
            with (tc.tile_pool(name="eB", bufs=2) as eB,
                  tc.tile_pool(name="pBacc", bufs=1, space="PSUM") as pBacc):
                for c in range(nch):
                    if c + 4 < nch:
                        _prefetch(c + 4)
                    gtb_c, gptb_c, gres_c = pf_tiles.pop(c)
                    ps_ov = [pBacc.tile([128, 512], F32, tag=f"ov{k}", bufs=1,
                                        name=f"ov{c}_{k}") for k in range(4)]
                    ps_r = pBacc.tile([128, 8], F32, tag="r", bufs=1, name=f"r{c}")
                    eps = []

                    def _scores(mt):
                        p_sc = pps.tile([128, 512], F32, tag="s", name=f"sc{c}_{mt}")
                        for dk in range(4):
                            nc.tensor.matmul(
                                p_sc[:], aq_sb[:, dk, mt * 128:(mt + 1) * 128],
                                gtb_c[:, :, dk, :], start=(dk == 0), stop=False)
                        for dk in range(4):
                            nc.tensor.matmul(
                                p_sc[:], agp_sb[:, dk, mt * 128:(mt + 1) * 128],
                                gptb_c[:, :, dk, :], start=False, stop=(dk == 3))
                        ep = eB.tile([128, 512], F32R, tag=f"ep{mt}", name=f"ep{c}_{mt}")
                        nc.scalar.activation(out=ep, in_=p_sc[:], func=EXP,
                                             bias=c_sb[:, mt * 2:mt * 2 + 1], scale=1.0)
                        eps.append(ep)

                    def _ov(mt):
                        for nk in range(4):
                            nc.tensor.matmul(
                                ps_ov[nk][:], eps[mt][:, nk * 128:(nk + 1) * 128],
                                v_sb[:, mt, :], start=(mt == 0), stop=(mt == 3))

                    # software-pipelined: scores(mt+1) is emitted before OV(mt)
                    # so the PE never waits on the exp of the tile it just
                    # produced.
                    _scores(0)
                    for mt in range(4):
                        if mt + 1 < 4:
                            _scores(mt + 1)
                        _ov(mt)
                    for nk in range(4):
                        for mt in range(4):
                            nc.tensor.matmul(
                                ps_r[:, nk * 2:nk * 2 + 2],
                                eps[mt][:, nk * 128:(nk + 1) * 128],
                                ones_c[:], start=(mt == 0), stop=(mt == 3))
                    rr_sb = eB.tile([128, 8], F32, tag="rr", name=f"rr{c}")
                    nc.vector.reciprocal(out=rr_sb, in_=ps_r[:])
                    s1_4 = eB.tile([128, 4], F32, tag="s1", name=f"s1{c}")
                    s2_4 = eB.tile([128, 4], F32, tag="s2", name=f"s2{c}")
                    pres = []
                    for nk in range(4):
                        pre = eB.tile([128, 512], F32, tag=f"pre{nk}",
                                      name=f"pre{c}_{nk}")
                        nc.vector.scalar_tensor_tensor(
                            out=pre, in0=ps_ov[nk][:], scalar=rr_sb[:, nk * 2:nk * 2 + 1],
                            in1=gres_c[:, nk, :], op0=MULT, op1=ADD,
                            accum_out=s1_4[:, nk:nk + 1])
                        pres.append(pre)
                        sqj = eB.tile([128, 512], F32, tag=f"sq{nk % 2}",
                                      name=f"sq{c}_{nk}")
                        nc.scalar.activation(out=sqj, in_=pre, func=SQUARE,
                                             accum_out=s2_4[:, nk:nk + 1])
                    # mu = s1/512 ; var = s2/512 - mu^2 ; rstd = exp(-.5*ln(var+eps))
                    mu4 = eB.tile([128, 4], F32, tag="mu", name=f"mu{c}")
                    var4 = eB.tile([128, 4], F32, tag="var", name=f"var{c}")
                    rstd4 = eB.tile([128, 4], F32, tag="rstd", name=f"rstd{c}")
                    nc.vector.tensor_scalar_mul(out=mu4, in0=s1_4, scalar1=1.0 / 512)
                    nc.vector.tensor_mul(out=var4, in0=mu4, in1=mu4)
                    nc.vector.scalar_tensor_tensor(
                        out=var4, in0=s2_4, scalar=1.0 / 512, in1=var4,
                        op0=MULT, op1=SUB)
                    nc.scalar.activation(out=rstd4, in_=var4, func=LN_F, bias=eps_sb)
                    nc.scalar.activation(out=rstd4, in_=rstd4, func=EXP,
                                         scale=nhalf_sb)
                    ob = eB.tile([128, 4, 512], F32, tag="ob", name=f"ob{c}")
                    out_r = out_d.rearrange("(c nk p) d -> c nk p d", p=128, nk=4)
                    for nk in range(4):
                        nc.vector.tensor_scalar(out=ob[:, nk, :], in0=pres[nk],
                                                scalar1=mu4[:, nk:nk + 1],
                                                scalar2=rstd4[:, nk:nk + 1],
                                                op0=SUB, op1=MULT)
                        if not gb_trivial:
                            nc.vector.tensor_mul(out=ob[:, nk, :], in0=ob[:, nk, :],
                                                 in1=gam_bc)
                            nc.vector.tensor_add(out=ob[:, nk, :], in0=ob[:, nk, :],
                                                 in1=bet_bc)
                        # per-nk store on the Act DGE queue: each slab leaves
                        # as soon as its LN finishes (shrinks the tail), and
                        # the sync queue stays a pure prefetch stream.
                        nc.scalar.dma_start(out=out_r[c, nk], in_=ob[:, nk, :])
            pf_ctx.__exit__(None, None, None)
    nc.compile()
    return nc


_CACHE = {}


def _get_nc(n_loc, n_cores, gb_trivial):
    key = (n_loc, n_cores, gb_trivial)
    if key not in _CACHE:
        _CACHE[key] = build(n_loc, n_cores, gb_trivial)
    return _CACHE[key]


def _packT(slab):
    """[n_loc, 512] f32 -> [128, nt, 4, 128] transposed pack: out[p,t,k,j] =
    slab[t*128+j, k*128+p]."""
    n_loc = slab.shape[0]
    return np.ascontiguousarray(
        slab.reshape(n_loc // 128, 128, 4, 128).transpose(3, 0, 2, 1))


def kernel(g, g_p, W, Wq, bq, Wk, bk, Wv, bv, Wgp, bgp, Wkp, bkp, gamma, beta,
           _trace=False):
    g = np.asarray(g, np.float32)
    g_p = np.asarray(g_p, np.float32)
    gamma = np.asarray(gamma, np.float32)
    beta = np.asarray(beta, np.float32)
    gb_trivial = bool(np.all(gamma == 1.0) and np.all(beta == 0.0))
    n = g.shape[0]
    n_loc = n // N_CORES
    nc = _get_nc(n_loc, N_CORES, gb_trivial)

    Ws = np.asarray(W, np.float32).reshape(M, D)
    Wk_ = np.asarray(Wk, np.float32)
    Wv_ = np.asarray(Wv, np.float32)
    shared = {
        # wsT[p,k,m] = Ws[m, k*128+p]
        "wsT": np.ascontiguousarray(Ws.reshape(M, 4, 128).transpose(2, 1, 0)),
        # wkT[p,t,d] = Wk[d, t*128+p]
        "wkT": np.ascontiguousarray(Wk_.T.reshape(4, 128, D).transpose(1, 0, 2)),
        "wvT": np.ascontiguousarray(Wv_.T.reshape(4, 128, D).transpose(1, 0, 2)),
        "wq": np.asarray(Wq, np.float32), "wgp": np.asarray(Wgp, np.float32),
        "wkp": np.asarray(Wkp, np.float32),
        "bq": np.asarray(bq, np.float32), "bk": np.asarray(bk, np.float32),
        "bv": np.asarray(bv, np.float32), "bgp": np.asarray(bgp, np.float32),
        "bkp": np.asarray(bkp, np.float32),
    }
    if not gb_trivial:
        shared["gamma"] = gamma
        shared["beta"] = beta
    in_maps = []
    for cid in range(N_CORES):
        sl = slice(cid * n_loc, (cid + 1) * n_loc)
        g_slab = np.ascontiguousarray(g[sl])
        gp_slab = np.ascontiguousarray(g_p[sl])
        gt32 = _packT(g_slab)
        gpt32 = _packT(gp_slab)
        in_maps.append({
            "gt32": gt32,
            "gpb": gp_slab.astype(bfloat16),
            "gtb": gt32.astype(bfloat16),
            "gptb": gpt32.astype(bfloat16),
            "gres": g_slab.astype(bfloat16),
            **shared,
        })
    res = bass_utils.run_bass_kernel_spmd(
        nc, in_maps, core_ids=list(range(N_CORES)), trace=_trace)
    out = np.concatenate([res.results[cid]["out"] for cid in range(N_CORES)], axis=0)
    if _trace:
        return out, res
    return out


# revision 38
# speedup vs baseline: 1.0199x; 1.0106x over previous
"""Trainium2 Bass kernel for nn_CrossAttention (N=65536 gaussians, M=512 tokens, D=512).

Runs SPMD on 8 NeuronCores; N sharded across cores.

Host marshalling supplies g/g_p in both natural layout (f32) and transposed
packed layout ([128, nt, 4, 128]; f32 for the pooling pass, bf16 for the
attention pass), plus pre-transposed weight packs.  This removes all on-device
PE transposes and their PSUM->SBUF copies.

Per core (n_loc rows):
  Phase A (pool): per 128-row tile, p_s = g @ Ws.T via host-gT stationary;
    et = exp(p_s - C_SHIFT); accumulate P.T = gp.T @ et and l = 1.T @ et in
    PSUM across all tiles.
  AllReduce (bf16) of (P.T || l); weight-prep matmuls (K0T, V, Aq0, Agp0,
    GqT, GgpT, c0, u0) are emitted after phase A so they execute during the
    collective.
  Fixups: kplT = P.T/l;  Aq = SCALE*Wq.T@K.T = Aq0 + GqT@kplT (stored bf16),
    likewise Agp; c = c0 + u0-part (score bias from bq+bgp; zero for the
    given inputs but handled generally).
  Phase B (attention): per 512-row chunk, scoresT = Aq.T@gT + Agp.T@gpT in
    bf16; ep = exp(scoresT + c); OV and rowsum r via PE; LN fused:
    pre = OV*(1/r) + g (scalar_tensor_tensor, accum -> mean),
    sumsq via activation Square accum, rstd = exp(-0.5*ln(var+eps)),
    out = (pre - mu)*rstd [*gamma + beta if non-trivial].
"""
import numpy as np
from ml_dtypes import bfloat16

import concourse.bass as bass
import concourse.tile as tile
from concourse import bacc, bass_isa, mybir, bass_utils

N_CORES = 8
N_FULL = 65536
D = 512
M = 512
SCALE = (D // 8) ** -0.5  # 0.125
LN_EPS = 1e-5
C_SHIFT = 115.0
F32 = mybir.dt.float32
F32R = mybir.dt.float32r
BF16 = mybir.dt.bfloat16
EXP = mybir.ActivationFunctionType.Exp
LN_F = mybir.ActivationFunctionType.Ln
SQUARE = mybir.ActivationFunctionType.Square
MULT = mybir.AluOpType.mult
ADD = mybir.AluOpType.add
SUB = mybir.AluOpType.subtract


def _bcast(ap, parts):
    """Partition-broadcast a [F]-shaped DRAM AP to [parts, F] for DMA."""
    return bass.AP(tensor=ap.tensor, offset=ap.offset, ap=[[0, parts], *ap.ap])


def build(n_loc=N_FULL // N_CORES, n_cores=N_CORES, gb_trivial=True):
    nt = n_loc // 128      # phase-A tiles of 128 rows
    nch = n_loc // 512     # phase-B chunks of 512 rows
    assert n_loc % 512 == 0

    nc = bacc.Bacc("TRN2", target_bir_lowering=False, debug=False, num_devices=n_cores)
    gt32_d = nc.dram_tensor("gt32", [128, nt, 4, 128], F32R, kind="ExternalInput").ap()
    gpb_d = nc.dram_tensor("gpb", [n_loc, D], BF16, kind="ExternalInput").ap()
    gtb_d = nc.dram_tensor("gtb", [128, nt, 4, 128], BF16, kind="ExternalInput").ap()
    gptb_d = nc.dram_tensor("gptb", [128, nt, 4, 128], BF16, kind="ExternalInput").ap()
    gres_d = nc.dram_tensor("gres", [n_loc, D], BF16, kind="ExternalInput").ap()
    wsT_d = nc.dram_tensor("wsT", [128, 4, D], F32R, kind="ExternalInput").ap()
    wkT_d = nc.dram_tensor("wkT", [128, 4, D], F32R, kind="ExternalInput").ap()
    wvT_d = nc.dram_tensor("wvT", [128, 4, D], F32R, kind="ExternalInput").ap()
    wq_d = nc.dram_tensor("wq", [D, D], F32R, kind="ExternalInput").ap()
    wgp_d = nc.dram_tensor("wgp", [D, D], F32R, kind="ExternalInput").ap()
    wkp_d = nc.dram_tensor("wkp", [D, D], F32R, kind="ExternalInput").ap()
    bq_d = nc.dram_tensor("bq", [D], F32, kind="ExternalInput").ap()
    bk_d = nc.dram_tensor("bk", [D], F32, kind="ExternalInput").ap()
    bv_d = nc.dram_tensor("bv", [D], F32, kind="ExternalInput").ap()
    bgp_d = nc.dram_tensor("bgp", [D], F32, kind="ExternalInput").ap()
    bkp_d = nc.dram_tensor("bkp", [D], F32, kind="ExternalInput").ap()
    if not gb_trivial:
        gam_d = nc.dram_tensor("gamma", [D], F32, kind="ExternalInput").ap()
        bet_d = nc.dram_tensor("beta", [D], F32, kind="ExternalInput").ap()
    out_d = nc.dram_tensor("out", [n_loc, D], F32, kind="ExternalOutput").ap()

    with tile.TileContext(nc) as tc:
        with (
            tc.tile_pool(name="wts", bufs=1) as wts,
            tc.tile_pool(name="ps", bufs=3, space="PSUM") as pps,
            tc.tile_pool(name="dram", bufs=1, space="DRAM") as dpool,
        ):
            # ---------- persistent tiles ----------
            # Pin the activation table to set 6 (natural_log_exp_and_others:
            # exp+ln+copy+square) so the table-load pass never alternates
            # between the exp-first and ln-first sets (1.28us per reload).
            nc.scalar.add_instruction(mybir.InstLoadActFuncSet(
                name=nc.get_next_instruction_name(),
                act_func_set_id=6, ins=[], outs=[]))
            ones_f = wts.tile([128, 128], F32)
            nc.vector.memset(ones_f, 1.0)
            ones_m = wts.tile([128, 128], F32R)
            nc.vector.tensor_copy(out=ones_m, in_=ones_f)
            ones_c = wts.tile([128, 2], F32R)
            nc.vector.tensor_copy(out=ones_c, in_=ones_f[:, 0:2])
            negc_sb = wts.tile([128, 1], F32)
            nc.vector.memset(negc_sb, -C_SHIFT)
            eps_sb = wts.tile([128, 1], F32)
            nc.vector.memset(eps_sb, LN_EPS)
            nhalf_sb = wts.tile([128, 1], F32)
            nc.vector.memset(nhalf_sb, -0.5)
            wsT = wts.tile([128, 4, D], F32R)
            for k in range(4):
                nc.sync.dma_start(out=wsT[:, k, :], in_=wsT_d[:, k, :])
            v_sb = wts.tile([128, 4, D], F32R)   # V [m-part, mt, d]
            aq_sb = wts.tile([128, 4, D], BF16)  # SCALE*Wq.T@K.T [d-part, dt, m]
            agp_sb = wts.tile([128, 4, D], BF16)
            c_sb = wts.tile([128, 8], F32)       # score bias c[m] as [m-part, (mt,2)]
            bv_bc = wts.tile([128, D], F32)
            nc.scalar.dma_start(out=bv_bc, in_=_bcast(bv_d, 128))
            if not gb_trivial:
                gam_bc = wts.tile([128, D], F32)
                bet_bc = wts.tile([128, D], F32)
                nc.scalar.dma_start(out=gam_bc, in_=_bcast(gam_d, 128))
                nc.scalar.dma_start(out=bet_bc, in_=_bcast(bet_d, 128))
            # weight/bias tiles persist in wts; their DMAs ride the Act-engine
            # DGE queue and are emitted a few tiles into phase A so the first
            # g-tile loads aren't bandwidth-starved.
            wkT = wts.tile([128, 4, D], F32R)
            wvT = wts.tile([128, 4, D], F32R)
            wq_n = wts.tile([128, 4, D], F32R)
            wgp_n = wts.tile([128, 4, D], F32R)
            wkp_n = wts.tile([128, 4, D], F32R)

            _wdma = [
                lambda: nc.scalar.dma_start(out=wkT, in_=wkT_d),
                lambda: nc.scalar.dma_start(out=wvT, in_=wvT_d),
                lambda: nc.scalar.dma_start(
                    out=wq_n, in_=wq_d.rearrange("(t p) d -> p t d", p=128)),
                lambda: nc.scalar.dma_start(
                    out=wgp_n, in_=wgp_d.rearrange("(t p) d -> p t d", p=128)),
                lambda: nc.scalar.dma_start(
                    out=wkp_n, in_=wkp_d.rearrange("(t p) d -> p t d", p=128)),
            ]
            bk_sb = wts.tile([128, 4], F32)
            bs_sb = wts.tile([128, 4], F32)
            bq_sb = wts.tile([128, 4], F32)
            bkp_sb = wts.tile([128, 4], F32)
            nc.scalar.dma_start(out=bk_sb, in_=bk_d.rearrange("(t p) -> p t", p=128))
            nc.scalar.dma_start(out=bq_sb, in_=bq_d.rearrange("(t p) -> p t", p=128))
            nc.scalar.dma_start(out=bs_sb, in_=bgp_d.rearrange("(t p) -> p t", p=128))
            nc.scalar.dma_start(out=bkp_sb,
                                in_=bkp_d.rearrange("(t p) -> p t", p=128))

            # warm the PE pstate during the initial DMA wait
            p_w = pps.tile([128, 128], F32, tag="s", name="p_warm")
            for w in range(24):
                nc.tensor.matmul(p_w[:], ones_m[:], ones_m[:],
                                 start=(w == 0), stop=(w == 23))

            # ---------- phase A: pooling partials ----------
            ctxA = tc.tile_pool(name="pAacc", bufs=1, space="PSUM")
            pAacc = ctxA.__enter__()
            ps_p = [pAacc.tile([128, 512], F32, tag=f"psp{i}", bufs=1,
                               name=f"ps_p{i}") for i in range(4)]
            l_acc = wts.tile([128, 512], F32)
            nc.vector.memset(l_acc, 0.0)
            with tc.tile_pool(name="sA", bufs=6) as sA:
                for i in range(nt):
                    if i >= 8 and i % 4 == 0 and _wdma:
                        _wdma.pop(0)()
                    gT_t = sA.tile([128, 4, 128], F32R, tag="gTA", name=f"gTA{i}")
                    gpn = sA.tile([128, D], BF16, tag="gpnA", name=f"gpnA{i}")
                    nc.sync.dma_start(out=gT_t, in_=gt32_d[:, i, :, :])
                    nc.sync.dma_start(out=gpn, in_=gpb_d[i * 128:(i + 1) * 128, :])
                    p_s = pps.tile([128, 512], F32, tag="s", name=f"psA{i}")
                    for dk in range(4):
                        nc.tensor.matmul(p_s[:], gT_t[:, dk, :], wsT[:, dk, :],
                                         start=(dk == 0), stop=(dk == 3))
                    et = sA.tile([128, 512], BF16, tag="etA", name=f"etA{i}")
                    nc.scalar.activation(out=et, in_=p_s[:], func=EXP,
                                         bias=negc_sb, scale=1.0)
                    for d2t in range(4):
                        nc.tensor.matmul(
                            ps_p[d2t][:], gpn[:, d2t * 128:(d2t + 1) * 128], et[:],
                            start=(i == 0), stop=(i == nt - 1))
                    # l += colsums(et) off the PE: partition-reduce on the Q7
                    # (Pool) engine, accumulate on the (idle) DVE.
                    l_i = sA.tile([128, 512], F32, tag="liA", name=f"liA{i}")
                    nc.gpsimd.partition_all_reduce(
                        l_i, et[:], channels=128, reduce_op=bass_isa.ReduceOp.add)
                    nc.vector.tensor_add(out=l_acc, in0=l_acc, in1=l_i)

            # ---------- all-reduce of (P.T || l), bf16 ----------
            # pf opens BEFORE w0 so its SBUF region does not overlap w0's
            # tiles; its prefetch DMAs (sync queue) can then run during the
            # collective without anti-dependency stalls.
            pf_ctx = tc.tile_pool(name="pf", bufs=4)
            pf = pf_ctx.__enter__()
            pf_tiles = {}

            def _prefetch(c):
                gtb_c = pf.tile([128, 4, 4, 128], BF16, tag="gtb", name=f"gtb{c}")
                gptb_c = pf.tile([128, 4, 4, 128], BF16, tag="gptb", name=f"gptb{c}")
                gres_c = pf.tile([128, 4, 512], BF16, tag="gres", name=f"gres{c}")
                nc.sync.dma_start(out=gtb_c, in_=gtb_d[:, 4 * c:4 * c + 4, :, :])
                nc.sync.dma_start(out=gptb_c, in_=gptb_d[:, 4 * c:4 * c + 4, :, :])
                nc.sync.dma_start(
                    out=gres_c,
                    in_=gres_d[c * 512:(c + 1) * 512, :].rearrange(
                        "(nk p) d -> p nk d", p=128))
                pf_tiles[c] = (gtb_c, gptb_c, gres_c)

            with tc.tile_pool(name="arp", bufs=1) as arp:
                pl_sb = arp.tile([128, 5, 512], BF16)
                for d2t in range(4):
                    if d2t % 2 == 0:
                        nc.vector.tensor_copy(out=pl_sb[:, d2t, :], in_=ps_p[d2t][:])
                    else:
                        nc.scalar.copy(out=pl_sb[:, d2t, :], in_=ps_p[d2t][:])
                nc.vector.tensor_copy(out=pl_sb[:, 4, :], in_=l_acc)
                ctxA.__exit__(None, None, None)
                # ReduceScatter + AllGather = AllReduce, but ~20% cheaper on
                # the interconnect (no duplicate-reduce traffic).
                ar_in = dpool.tile([128, 5 * 512], BF16)
                rs_out = dpool.tile([128, 5 * 512 // n_cores], BF16)
                ar_out = dpool.tile([128, 5 * 512], BF16, addr_space="Shared")
                nc.sync.dma_start(out=ar_in[:],
                                  in_=pl_sb[:].rearrange("p a b -> p (a b)"))
                nc.gpsimd.collective_compute(
                    "ReduceScatter", mybir.AluOpType.add,
                    replica_groups=[list(range(n_cores))],
                    ins=[ar_in.opt()], outs=[rs_out.opt()])
                nc.gpsimd.collective_compute(
                    "AllGather", mybir.AluOpType.bypass,
                    replica_groups=[list(range(n_cores))],
                    ins=[rs_out.opt()], outs=[ar_out.opt()])
                # phase-B prefetch begins immediately; these loads only await
                # free pf buffers, so they overlap the collective.
                for c in range(min(4, nch)):
                    _prefetch(c)

                # ---------- phase 0 weight prep (overlaps the collective) ----
                with tc.tile_pool(name="w0", bufs=1) as w0:
                    nc.vector.tensor_add(out=bs_sb, in0=bs_sb, in1=bq_sb)
                    # K.T = Wk@Ws.T + (bk + bkp) + Wkp@kpool.T -> fold bkp into K0T bias
                    nc.vector.tensor_add(out=bk_sb, in0=bk_sb, in1=bkp_sb)

                    k0T = w0.tile([128, 4, D], F32R)
                    aq0 = w0.tile([128, 4, D], F32)
                    agp0 = w0.tile([128, 4, D], F32)
                    gqT = w0.tile([128, 4, D], F32R)
                    ggpT = w0.tile([128, 4, D], F32R)
                    c0_sb = w0.tile([128, 8], F32)
                    u0c2 = w0.tile([128, 4, 2], F32R)
                    # K0T[d, m] = Wk @ Ws.T + bk'
                    for dt in range(4):
                        p_k = pps.tile([128, 512], F32, tag="s", name=f"pk{dt}")
                        for di in range(4):
                            nc.tensor.matmul(p_k[:], wkT[:, di, dt * 128:(dt + 1) * 128],
                                             wsT[:, di, :], start=(di == 0), stop=(di == 3))
                        nc.vector.tensor_scalar_add(out=k0T[:, dt, :], in0=p_k[:],
                                                    scalar1=bk_sb[:, dt:dt + 1])
                    # V[m, d] = Ws @ Wv.T + bv
                    for mt in range(4):
                        p_v = pps.tile([128, 512], F32, tag="s", name=f"pv{mt}")
                        for di in range(4):
                            nc.tensor.matmul(p_v[:], wsT[:, di, mt * 128:(mt + 1) * 128],
                                             wvT[:, di, :], start=(di == 0), stop=(di == 3))
                        nc.vector.tensor_add(out=v_sb[:, mt, :], in0=p_v[:], in1=bv_bc)
                    # Aq0 = SCALE*Wq.T@K0T ; Agp0 likewise
                    for w_nat, dst in [(wq_n, aq0), (wgp_n, agp0)]:
                        for dt in range(4):
                            p_a = pps.tile([128, 512], F32, tag="s",
                                           name=f"pa_{dst.tensor.name}_{dt}")
                            for di in range(4):
                                nc.tensor.matmul(
                                    p_a[:], w_nat[:, di, dt * 128:(dt + 1) * 128],
                                    k0T[:, di, :], start=(di == 0), stop=(di == 3))
                            nc.scalar.mul(out=dst[:, dt, :], in_=p_a[:], mul=SCALE)
                    # GqT = SCALE*(Wkp.T @ Wq) ; GgpT likewise
                    for w_nat, dst in [(wq_n, gqT), (wgp_n, ggpT)]:
                        for dt in range(4):
                            p_gq = pps.tile([128, 512], F32, tag="s",
                                            name=f"pg_{dst.tensor.name}_{dt}")
                            for di in range(4):
                                nc.tensor.matmul(
                                    p_gq[:], wkp_n[:, di, dt * 128:(dt + 1) * 128],
                                    w_nat[:, di, :], start=(di == 0), stop=(di == 3))
                            nc.scalar.mul(out=dst[:, dt, :], in_=p_gq[:], mul=SCALE)
                    # c0[m] = SCALE*(bq+bgp)@K0T ; u0 = SCALE*Wkp.T@(bq+bgp)
                    bsr2 = w0.tile([128, 4, 2], F32R)
                    nc.vector.tensor_copy(out=bsr2[:, :, 0], in_=bs_sb)
                    nc.vector.tensor_copy(out=bsr2[:, :, 1], in_=bs_sb)
                    ctx0 = tc.tile_pool(name="p0acc", bufs=1, space="PSUM")
                    p0acc = ctx0.__enter__()
                    p_c0 = p0acc.tile([128, 8], F32, tag="pc0", bufs=1, name="p_c0")
                    p_u0 = p0acc.tile([128, 8], F32, tag="pu0", bufs=1, name="p_u0")
                    for mt in range(4):
                        for di in range(4):
                            nc.tensor.matmul(
                                p_c0[:, mt * 2:mt * 2 + 2],
                                k0T[:, di, mt * 128:(mt + 1) * 128],
                                bsr2[:, di, :], start=(di == 0), stop=(di == 3))
                            nc.tensor.matmul(
                                p_u0[:, mt * 2:mt * 2 + 2],
                                wkp_n[:, di, mt * 128:(mt + 1) * 128],
                                bsr2[:, di, :], start=(di == 0), stop=(di == 3))
                    nc.scalar.mul(out=c0_sb, in_=p_c0[:], mul=SCALE)
                    nc.scalar.mul(out=u0c2.rearrange("p a b -> p (a b)"), in_=p_u0[:],
                                  mul=SCALE)
                    ctx0.__exit__(None, None, None)

                    # ---------- post-collective fixups ----------
                    # plr rides the Act-engine DGE queue (idle here), keeping
                    # the sync queue free for phase-B prefetch.
                    plr_sb = w0.tile([128, 5, 512], BF16)
                    nc.scalar.dma_start(out=plr_sb,
                                        in_=ar_out[:].rearrange("p (a b) -> p a b", a=5))
                    lr_sb = w0.tile([128, 512], F32)
                    nc.vector.reciprocal(out=lr_sb, in_=plr_sb[:, 4, :])
                    kplT = w0.tile([128, 4, D], F32R)
                    for dint in range(4):
                        nc.vector.tensor_mul(out=kplT[:, dint, :],
                                             in0=plr_sb[:, dint, :], in1=lr_sb)
                    for gT_w, base, dst in [(gqT, aq0, aq_sb), (ggpT, agp0, agp_sb)]:
                        for dt in range(4):
                            p_aq = pps.tile([128, 512], F32, tag="s",
                                            name=f"paq_{dst.tensor.name}_{dt}")
                            for di in range(4):
                                nc.tensor.matmul(
                                    p_aq[:], gT_w[:, di, dt * 128:(dt + 1) * 128],
                                    kplT[:, di, :], start=(di == 0), stop=(di == 3))
                            nc.vector.tensor_add(out=dst[:, dt, :], in0=base[:, dt, :],
                                                 in1=p_aq[:])
                    ctxP = tc.tile_pool(name="pPacc", bufs=1, space="PSUM")
                    pPacc = ctxP.__enter__()
                    p_cp = pPacc.tile([128, 8], F32, tag="pcp", bufs=1, name="p_cp")
                    for mt in range(4):
                        for di in range(4):
                            nc.tensor.matmul(
                                p_cp[:, mt * 2:mt * 2 + 2],
                                kplT[:, di, mt * 128:(mt + 1) * 128],
                                u0c2[:, di, :], start=(di == 0), stop=(di == 3))
                    nc.vector.tensor_add(out=c_sb, in0=c0_sb, in1=p_cp[:])
                    ctxP.__exit__(None, None, None)

            # ---------- phase B: attention ----------
            with (tc.tile_pool(name="eB", bufs=2) as eB,
                  tc.tile_pool(name="pBacc", bufs=1, space="PSUM") as pBacc):
                for c in range(nch):
                    if c + 4 < nch:
                        _prefetch(c + 4)
                    gtb_c, gptb_c, gres_c = pf_tiles.pop(c)
                    ps_ov = [pBacc.tile([128, 512], F32, tag=f"ov{k}", bufs=1,
                                        name=f"ov{c}_{k}") for k in range(4)]
                    ps_r = pBacc.tile([128, 8], F32, tag="r", bufs=1, name=f"r{c}")
                    eps = []

                    def _scores(mt):
                        p_sc = pps.tile([128, 512], F32, tag="s", name=f"sc{c}_{mt}")
                        for dk in range(4):
                            nc.tensor.matmul(
                                p_sc[:], aq_sb[:, dk, mt * 128:(mt + 1) * 128],
                                gtb_c[:, :, dk, :], start=(dk == 0), stop=False)
                        for dk in range(4):
                            nc.tensor.matmul(
                                p_sc[:], agp_sb[:, dk, mt * 128:(mt + 1) * 128],
                                gptb_c[:, :, dk, :], start=False, stop=(dk == 3))
                        ep = eB.tile([128, 512], F32R, tag=f"ep{mt}", name=f"ep{c}_{mt}")
                        nc.scalar.activation(out=ep, in_=p_sc[:], func=EXP,
                                             bias=c_sb[:, mt * 2:mt * 2 + 1], scale=1.0)
                        eps.append(ep)

                    def _ov(mt):
                        for nk in range(4):
                            nc.tensor.matmul(
                                ps_ov[nk][:], eps[mt][:, nk * 128:(nk + 1) * 128],
                                v_sb[:, mt, :], start=(mt == 0), stop=(mt == 3))
                            nc.tensor.matmul(
                                ps_r[:, nk * 2:nk * 2 + 2],
                                eps[mt][:, nk * 128:(nk + 1) * 128],
                                ones_c[:], start=(mt == 0), stop=(mt == 3))

                    # software-pipelined: scores(mt+1) is emitted before OV(mt)
                    # so the PE never waits on the exp of the tile it just
                    # produced.
                    _scores(0)
                    for mt in range(4):
                        if mt + 1 < 4:
                            _scores(mt + 1)
                        _ov(mt)
                    rr_sb = eB.tile([128, 8], F32, tag="rr", name=f"rr{c}")
                    nc.vector.reciprocal(out=rr_sb, in_=ps_r[:])
                    s1_4 = eB.tile([128, 4], F32, tag="s1", name=f"s1{c}")
                    s2_4 = eB.tile([128, 4], F32, tag="s2", name=f"s2{c}")
                    pres = []
                    for nk in range(4):
                        pre = eB.tile([128, 512], F32, tag=f"pre{nk}",
                                      name=f"pre{c}_{nk}")
                        nc.vector.scalar_tensor_tensor(
                            out=pre, in0=ps_ov[nk][:], scalar=rr_sb[:, nk * 2:nk * 2 + 1],
                            in1=gres_c[:, nk, :], op0=MULT, op1=ADD,
                            accum_out=s1_4[:, nk:nk + 1])
                        pres.append(pre)
                        sqj = eB.tile([128, 512], F32, tag=f"sq{nk % 2}",
                                      name=f"sq{c}_{nk}")
                        nc.scalar.activation(out=sqj, in_=pre, func=SQUARE,
                                             accum_out=s2_4[:, nk:nk + 1])
                    # mu = s1/512 ; var = s2/512 - mu^2 ; rstd = exp(-.5*ln(var+eps))
                    mu4 = eB.tile([128, 4], F32, tag="mu", name=f"mu{c}")
                    var4 = eB.tile([128, 4], F32, tag="var", name=f"var{c}")
                    rstd4 = eB.tile([128, 4], F32, tag="rstd", name=f"rstd{c}")
                    nc.vector.tensor_scalar_mul(out=mu4, in0=s1_4, scalar1=1.0 / 512)
                    nc.vector.tensor_mul(out=var4, in0=mu4, in1=mu4)
                    nc.vector.scalar_tensor_tensor(
                        out=var4, in0=s2_4, scalar=1.0 / 512, in1=var4,
                        op0=MULT, op1=SUB)
                    nc.scalar.activation(out=rstd4, in_=var4, func=LN_F, bias=eps_sb)
                    nc.scalar.activation(out=rstd4, in_=rstd4, func=EXP,
                                         scale=nhalf_sb)
                    ob = eB.tile([128, 4, 512], F32, tag="ob", name=f"ob{c}")
                    out_r = out_d.rearrange("(c nk p) d -> c nk p d", p=128, nk=4)
                    for nk in range(4):
                        nc.vector.tensor_scalar(out=ob[:, nk, :], in0=pres[nk],
                                                scalar1=mu4[:, nk:nk + 1],
                                                scalar2=rstd4[:, nk:nk + 1],
                                                op0=SUB, op1=MULT)
                        if not gb_trivial:
                            nc.vector.tensor_mul(out=ob[:, nk, :], in0=ob[:, nk, :],
                                                 in1=gam_bc)
                            nc.vector.tensor_add(out=ob[:, nk, :], in0=ob[:, nk, :],
                                                 in1=bet_bc)
                        # per-nk store on the Act DGE queue: each slab leaves
                        # as soon as its LN finishes (shrinks the tail), and
                        # the sync queue stays a pure prefetch stream.
                        nc.scalar.dma_start(out=out_r[c, nk], in_=ob[:, nk, :])
            pf_ctx.__exit__(None, None, None)
    nc.compile()
    return nc


_CACHE = {}


def _get_nc(n_loc, n_cores, gb_trivial):
    key = (n_loc, n_cores, gb_trivial)
    if key not in _CACHE:
        _CACHE[key] = build(n_loc, n_cores, gb_trivial)
    return _CACHE[key]


def _packT(slab):
    """[n_loc, 512] f32 -> [128, nt, 4, 128] transposed pack: out[p,t,k,j] =
    slab[t*128+j, k*128+p]."""
    n_loc = slab.shape[0]
    return np.ascontiguousarray(
        slab.reshape(n_loc // 128, 128, 4, 128).transpose(3, 0, 2, 1))


def kernel(g, g_p, W, Wq, bq, Wk, bk, Wv, bv, Wgp, bgp, Wkp, bkp, gamma, beta,
           _trace=False):
    g = np.asarray(g, np.float32)
    g_p = np.asarray(g_p, np.float32)
    gamma = np.asarray(gamma, np.float32)
    beta = np.asarray(beta, np.float32)
    gb_trivial = bool(np.all(gamma == 1.0) and np.all(beta == 0.0))
    n = g.shape[0]
    n_loc = n // N_CORES
    nc = _get_nc(n_loc, N_CORES, gb_trivial)

    Ws = np.asarray(W, np.float32).reshape(M, D)
    Wk_ = np.asarray(Wk, np.float32)
    Wv_ = np.asarray(Wv, np.float32)
    shared = {
        # wsT[p,k,m] = Ws[m, k*128+p]
        "wsT": np.ascontiguousarray(Ws.reshape(M, 4, 128).transpose(2, 1, 0)),
        # wkT[p,t,d] = Wk[d, t*128+p]
        "wkT": np.ascontiguousarray(Wk_.T.reshape(4, 128, D).transpose(1, 0, 2)),
        "wvT": np.ascontiguousarray(Wv_.T.reshape(4, 128, D).transpose(1, 0, 2)),
        "wq": np.asarray(Wq, np.float32), "wgp": np.asarray(Wgp, np.float32),
        "wkp": np.asarray(Wkp, np.float32),
        "bq": np.asarray(bq, np.float32), "bk": np.asarray(bk, np.float32),
        "bv": np.asarray(bv, np.float32), "bgp": np.asarray(bgp, np.float32),
        "bkp": np.asarray(bkp, np.float32),
    }
    if not gb_trivial:
        shared["gamma"] = gamma
        shared["beta"] = beta
    in_maps = []
    for cid in range(N_CORES):
        sl = slice(cid * n_loc, (cid + 1) * n_loc)
        g_slab = np.ascontiguousarray(g[sl])
        gp_slab = np.ascontiguousarray(g_p[sl])
        gt32 = _packT(g_slab)
        gpt32 = _packT(gp_slab)
        in_maps.append({
            "gt32": gt32,
            "gpb": gp_slab.astype(bfloat16),
            "gtb": gt32.astype(bfloat16),
            "gptb": gpt32.astype(bfloat16),
            "gres": g_slab.astype(bfloat16),
            **shared,
        })
    res = bass_utils.run_bass_kernel_spmd(
        nc, in_maps, core_ids=list(range(N_CORES)), trace=_trace)
    out = np.concatenate([res.results[cid]["out"] for cid in range(N_CORES)], axis=0)
    if _trace:
        return out, res
    return out
